# revision 80
# baseline (speedup 1.0000x reference)
"""Trainium2 Bass kernel for DescriptorMatcher (mutual nearest neighbor matching).

Problem: given desc0 [B,N,D], desc1 [B,M,D] (B=4, N=M=8192, D=128, fp32):
    sim     = desc0 @ desc1^T                      [B,N,M]
    score0  = max_m sim                            [B,N]
    match01 = argmax_m sim                         [B,N]
    match10 = argmax_n sim                         [B,M]
    valid   = (match10[match01[n]] == n) & (score0 > 0.1)
returns (match01, score0, valid).

Key reformulation: the mutual check never needs match10 indices:
    match10[match01[n]] == n  <=>  score0[n] == colmax[match01[n]]
when all maxima are taken over the SAME rounded values. All on-device
max bookkeeping runs on fp16-rounded copies of the PSUM values: fp16
rounding is monotonic, so max(fp16(v)) == fp16(max(v)) and the
equality trick holds bit-exactly in the fp16 value system. Matmul
inputs ship as bf16 (halves the bandwidth-bound input DMA; the PE
accumulates bf16 products in fp32). Measured argmax flips vs the fp32
reference (bf16 inputs + fp16 value pipeline): 4.8e-3 (gate 2e-2).

Engine constraints that shape the design (walrus BIR verifier): GPSIMD
(Pool) cannot access PSUM and has no tensor_tensor/tensor_scalar ucode
(only partition_all_reduce/memset/ISA lib); TRN2 matmul output must be
fp32; ACT accum is sum-only; DVE fp16 runs tensor_scalar at 4x and
tensor_tensor at 2x (SBUF operands).

Phase 1 (per core = batch x row-half), per 128-row tile [128 x 8192]:
    PE:   16 fp32r matmuls; chunks 0-11 into [128,1536] PSUM tiles
          (bufs=2), chunks 12-15 into [128,512] PSUM tiles (bufs=2)
    ACT:  4x 1536-wide + 1x 512-wide PSUM->SBUF fp16 copies (chunks
          0-12) — ACT is the bottleneck engine at ~100% busy
    DVE:  fused TS copy+chunk-max chunks 13-15 (PSUM src), 13 junk-TS
          chunk-maxes on fp16 (4x mode), colacc[0:CD] = max(.., row)
          (emitted one tile late so pieces are always ready)
    Pool: per-tile partition_all_reduce of row[CD:M] (2 pieces),
          DMA'd out per tile; host folds the 32 per-tile partials
Outputs cm [128, 32*16] fp16 chunk maxima, colacc [128, CD] fp16 and
colp [32, M-CD]; the final column reduces happen on the host.

Phase 2: rows grouped by winning chunk (host); recompute sim[:, chunk]
with identically-laid-out bf16 matmuls -> bit-identical PSUM -> ACT
fp16 copy -> max_index(cm_value, chunk_fp16) = exact first-occurrence
within-chunk argmax. match01 = chunk*512 + within. 36 subtiles with
slot capacities (640x4, 512x4); the host assigns the largest chunk-
groups to the big slots by permuting bt2/at2 slot contents.

Rows overflowing a slot's capacity fall back to a host recompute.
"""

import numpy as np

import concourse.bass as bass  # noqa: F401  (bass must import before tile)
import concourse.mybir as mybir
import concourse.tile as tile
from concourse import bacc, bass_isa

B, N, M, D = 4, 8192, 8192, 128
NCORES = 8
HALF = N // 2          # rows per phase-1 core
NT = HALF // 128       # 32 n-tiles per core
CW = 512               # chunk width (phase-2 recompute width)
NCHUNK = M // CW       # 16 chunks per row
CD = 3712              # colacc columns on DVE; Pool reduces [CD, M) per tile
GRP = NCHUNK // 2      # 8 chunk-groups per phase-2 core
# Per-core slot capacities (rows) for the 8 chunk-groups: the host assigns
# the largest groups to the big slots (counts are ~N(512, 22), max observed
# 580 on the reference inputs); rows beyond a slot's capacity fall back to
# a host-side recompute.
SLOTS = (640, 640, 640, 640, 512, 512, 512, 512)
SLOT_BASE = tuple(int(x) for x in np.cumsum((0,) + SLOTS)[:-1])
NROWS2 = sum(SLOTS)    # 4608 phase-2 row slots per core
NST = NROWS2 // 128    # 36 phase-2 sub-tiles


def _build1():
    f32 = mybir.dt.float32
    f32r = mybir.dt.float32r
    f16 = mybir.dt.float16
    nc = bacc.Bacc("TRN2", target_bir_lowering=False, debug=False,
                   num_devices=NCORES)
    bf16 = mybir.dt.bfloat16
    at = nc.dram_tensor("at", [D, HALF], bf16, kind="ExternalInput").ap()
    bt = nc.dram_tensor("bt", [D, M], bf16, kind="ExternalInput").ap()
    cm_o = nc.dram_tensor("cm", [128, NT * NCHUNK], f16,
                          kind="ExternalOutput").ap()
    colacc_o = nc.dram_tensor("colacc", [128, CD], f16,
                              kind="ExternalOutput").ap()
    colp_o = nc.dram_tensor("colp", [NT, M - CD], f16,
                            kind="ExternalOutput").ap()

    with tile.TileContext(nc) as tc:
        with tc.tile_pool(name="big", bufs=1) as big, \
             tc.tile_pool(name="rows", bufs=5) as rows, \
             tc.tile_pool(name="dmy", bufs=8) as dmy, \
             tc.tile_pool(name="cps", bufs=3) as cps, \
             tc.tile_pool(name="psa", bufs=2, space="PSUM") as psa, \
             tc.tile_pool(name="psp", bufs=2, space="PSUM") as psp:
            atb = big.tile([128, HALF], bf16, name="atb")
            btb = big.tile([128, M], bf16, name="btb")
            # tile 0 needs at[:, 0:128] and then bt chunks in matmul order;
            # front-load tiny slices on BOTH DGE queues so the PE starts ASAP
            nc.scalar.dma_start(atb[:, 0:128], at[:, 0:128])
            nc.sync.dma_start(btb[:, 0:512], bt[:, 0:512])
            nc.sync.dma_start(btb[:, 512:1024], bt[:, 512:1024])
            nc.scalar.dma_start(btb[:, 1024:2048], bt[:, 1024:2048])
            # rest of bt in wide transfers so tile 0's tail chunks aren't
            # starved behind a long descriptor queue; at after bt.
            for c in range(2048, M, 3072):
                w = min(3072, M - c)
                nc.sync.dma_start(btb[:, c:c + w], bt[:, c:c + w])
            nc.sync.dma_start(atb[:, 128:1024], at[:, 128:1024])
            for c in range(1024, HALF, 3072):
                w = min(3072, HALF - c)
                nc.sync.dma_start(atb[:, c:c + w], at[:, c:c + w])
            cm_all = big.tile([128, NT * NCHUNK], f16, name="cm_all")
            colacc = big.tile([128, M], f16, name="colacc")

            # colacc piece boundaries (moderate quanta: overhead vs blocking)
            dve_cuts = [0, CD]

            def colacc_update(tp, prow):
                """Column-max bookkeeping for row(tp). DVE accumulates
                colacc = max(colacc, row) on [0:CD]; Pool (which cannot
                read PSUM and has no tensor_tensor ucode) instead does a
                per-tile 128-partition max-reduce of row[CD:M], DMA'd out
                for the host to fold over tiles. Emitted one tile LATE so
                the pieces are ready when the engine queues reach them."""
                cuts = dve_cuts
                for lo, hi in zip(cuts[:-1], cuts[1:]):
                    if tp == 0:
                        nc.vector.tensor_copy(colacc[:, lo:hi], prow[:, lo:hi])
                    else:
                        nc.vector.tensor_tensor(colacc[:, lo:hi],
                                                colacc[:, lo:hi],
                                                prow[:, lo:hi],
                                                op=mybir.AluOpType.max)
                    if tp == NT - 1:
                        nc.sync.dma_start(colacc_o[:, lo:hi],
                                          colacc[:, lo:hi])
                # two pieces: the first needs only earlier chunks, so it
                # starts before the tail chunks land (shortens the drain)
                cp = cps.tile([128, M - CD], f16, tag="cp", name="cp")
                pw = (M - CD) // 2
                for q in range(2):
                    lo = CD + q * pw
                    hi = M if q == 1 else lo + pw
                    nc.gpsimd.partition_all_reduce(
                        cp[:, lo - CD:hi - CD], prow[:, lo:hi], channels=128,
                        reduce_op=bass_isa.ReduceOp.max)
                nc.sync.dma_start(colp_o[tp:tp + 1, :], cp[0:1, :])

            def emit_pa(t, h, row, cmt):
                pa = psa.tile([128, 1536], f32, tag="pa", name="pa")
                for j in range(3):
                    mlo = h * 1536 + j * 512
                    nc.tensor.matmul(pa[:, j * 512:(j + 1) * 512],
                                     atb[:, t * 128:(t + 1) * 128],
                                     btb[:, mlo:mlo + 512],
                                     start=True, stop=True)
                # PSUM escape: fp32 -> fp16 on ACT
                if t == 0 and h == 0:
                    for j in range(3):
                        nc.scalar.copy(row[:, j * 512:(j + 1) * 512],
                                       pa[:, j * 512:(j + 1) * 512])
                else:
                    nc.scalar.copy(row[:, h * 1536:(h + 1) * 1536], pa[:])
                # chunk maxima for ACT-copied chunks (DVE 4x mode);
                # junk output so colacc only depends on copies.
                dj = dmy.tile([128, 1536], f16, tag="dmy", name="dmy")
                for j in range(3):
                    ch = h * 3 + j
                    nc.vector.tensor_scalar(
                        dj[:, j * CW:(j + 1) * CW],
                        row[:, ch * CW:(ch + 1) * CW], 1.0, None,
                        op0=mybir.AluOpType.mult,
                        op1=mybir.AluOpType.max,
                        accum_out=cmt[:, ch:ch + 1])

            def emit_psp(t, row, cmt):
                for ch in range(12, 16):
                    pp = psp.tile([128, CW], f32, tag="pp", name="pp")
                    nc.tensor.matmul(pp[:],
                                     atb[:, t * 128:(t + 1) * 128],
                                     btb[:, ch * CW:(ch + 1) * CW],
                                     start=True, stop=True)
                    if ch == 12:
                        # chunk 12: ACT copy + DVE junk-TS max
                        nc.scalar.copy(row[:, 12 * CW:13 * CW], pp[:])
                        dj = dmy.tile([128, 1024], f16, tag="dmy", name="dmy")
                        nc.vector.tensor_scalar(
                            dj[:, 0:CW], row[:, 12 * CW:13 * CW], 1.0, None,
                            op0=mybir.AluOpType.mult,
                            op1=mybir.AluOpType.max,
                            accum_out=cmt[:, 12:13])
                    else:
                        # chunks 13-15: DVE-fused copy+chunk-max (PSUM src)
                        nc.vector.tensor_scalar(
                            row[:, ch * CW:(ch + 1) * CW], pp[:],
                            1.0, None,
                            op0=mybir.AluOpType.mult,
                            op1=mybir.AluOpType.max,
                            accum_out=cmt[:, ch:ch + 1])

            prev_row = None
            for t in range(NT):
                row = rows.tile([128, M], f16, tag="row", name="row")
                cmt = cm_all[:, t * NCHUNK:(t + 1) * NCHUNK]
                # Chunks 0-11: PE fills [128,1024] PSUM tiles (bufs=3, so
                # PE runs ahead of the ACT escape copies). GPSIMD cannot
                # read PSUM, so the remaining chunks 12-15 go through psp
                # tiles: chunk 12 ACT-copied, 13-15 DVE-fused copy+max.
                # Last tile: tail chunks first so the final colacc and
                # partition-reduce pieces overlap the last ACT copies.
                if t == NT - 1:
                    emit_psp(t, row, cmt)
                for h in range(4):
                    emit_pa(t, h, row, cmt)
                if t != NT - 1:
                    emit_psp(t, row, cmt)
                # delayed colacc update for the previous tile
                if prev_row is not None:
                    colacc_update(t - 1, prev_row)
                prev_row = row
                # stagger the (tiny) chunk-maxima output
                if t % 8 == 7 and t != NT - 1:
                    nc.sync.dma_start(
                        cm_o[:, (t - 7) * NCHUNK:(t + 1) * NCHUNK],
                        cm_all[:, (t - 7) * NCHUNK:(t + 1) * NCHUNK])
            colacc_update(NT - 1, prev_row)
            nc.sync.dma_start(cm_o[:, (NT - 8) * NCHUNK:],
                              cm_all[:, (NT - 8) * NCHUNK:])
    nc.compile()
    return nc


def _build2():
    f32, f32r, u32 = mybir.dt.float32, mybir.dt.float32r, mybir.dt.uint32
    f16 = mybir.dt.float16
    nc = bacc.Bacc("TRN2", target_bir_lowering=False, debug=False,
                   num_devices=NCORES)
    bf16 = mybir.dt.bfloat16
    at2 = nc.dram_tensor("at2", [D, NROWS2], bf16, kind="ExternalInput").ap()
    bt2 = nc.dram_tensor("bt2", [D, M // 2], bf16, kind="ExternalInput").ap()
    sg = nc.dram_tensor("sg", [128, NST * 8], f16, kind="ExternalInput").ap()
    idx_o = nc.dram_tensor("idx", [128, NST * 8], u32, kind="ExternalOutput").ap()
    with tile.TileContext(nc) as tc:
        with tc.tile_pool(name="big", bufs=1) as big, \
             tc.tile_pool(name="stg", bufs=4) as stg, \
             tc.tile_pool(name="ps", bufs=4, space="PSUM") as ps:
            a2b = big.tile([128, NROWS2], bf16, name="a2b")
            b2b = big.tile([128, M // 2], bf16, name="b2b")
            sgb = big.tile([128, NST * 8], f16, name="sgb")
            # first slices on both DGE queues in parallel
            nc.scalar.dma_start(a2b[:, 0:128], at2[:, 0:128])
            nc.sync.dma_start(b2b[:, 0:512], bt2[:, 0:512])
            nc.scalar.dma_start(sgb[:], sg[:])
            # interleave so group 0's matmuls start before all input lands
            na = (NROWS2 + 1023) // 1024
            nb = (M // 2) // 1024
            for i in range(max(na, nb)):
                if i < na:
                    c = i * 1024
                    lo = 128 if i == 0 else 0
                    w = min(1024, NROWS2 - c)
                    nc.sync.dma_start(a2b[:, c + lo:c + w],
                                      at2[:, c + lo:c + w])
                if i < nb:
                    c = i * 1024
                    lo = 512 if i == 0 else 0
                    nc.sync.dma_start(b2b[:, c + lo:c + 1024],
                                      bt2[:, c + lo:c + 1024])
            idx8 = big.tile([128, NST * 8], u32, name="idx8")
            st = -1
            for g in range(GRP):
                for k in range(SLOTS[g] // 128):
                    st += 1
                    pt = ps.tile([128, CW], f32, tag="pt", name="pt")
                    nc.tensor.matmul(pt[:],
                                     a2b[:, st * 128:(st + 1) * 128],
                                     b2b[:, g * CW:(g + 1) * CW],
                                     start=True, stop=True)
                    ch = stg.tile([128, CW], f16, tag="ch", name="ch")
                    nc.scalar.copy(ch[:], pt[:])
                    nc.vector.max_index(idx8[:, st * 8:(st + 1) * 8],
                                        sgb[:, st * 8:(st + 1) * 8], ch[:])
            nc.sync.dma_start(idx_o[:, 0:(NST - 1) * 8],
                              idx8[:, 0:(NST - 1) * 8])
            nc.sync.dma_start(idx_o[:, (NST - 1) * 8:],
                              idx8[:, (NST - 1) * 8:])
    nc.compile()
    return nc


_cached = None


def _make_exec(nc):
    import jax
    from jax.sharding import Mesh, PartitionSpec
    from jax.experimental.shard_map import shard_map
    from concourse import bass2jax
    from concourse.bass2jax import _bass_exec_p

    partition_name = nc.partition_id_tensor.name if nc.partition_id_tensor else None
    in_names, out_names, out_avals, out_shapes = [], [], [], []
    for alloc in nc.m.functions[0].allocations:
        if not isinstance(alloc, mybir.MemoryLocationSet):
            continue
        name = alloc.memorylocations[0].name
        if alloc.kind == "ExternalInput":
            if name != partition_name:
                in_names.append(name)
        elif alloc.kind == "ExternalOutput":
            shape = tuple(alloc.tensor_shape)
            dtype = mybir.dt.np(alloc.dtype)
            out_names.append(name)
            out_shapes.append((shape, dtype))
            out_avals.append(jax.core.ShapedArray(shape, dtype))
    n_params = len(in_names)
    n_outs = len(out_names)
    all_in_names = in_names + out_names
    if partition_name is not None:
        all_in_names = all_in_names + [partition_name]

    def _body(*args):
        operands = list(args)
        if partition_name is not None:
            operands.append(bass2jax.partition_id_tensor())
        outs = _bass_exec_p.bind(
            *operands, out_avals=tuple(out_avals), in_names=tuple(all_in_names),
            out_names=tuple(out_names), lowering_input_output_aliases=(),
            sim_require_finite=True, sim_require_nnan=True, nc=nc)
        return tuple(outs)

    devices = jax.devices()[:NCORES]
    mesh = Mesh(np.asarray(devices), ("core",))
    in_specs = (PartitionSpec("core"),) * (n_params + n_outs)
    out_specs = (PartitionSpec("core"),) * n_outs
    fn = jax.jit(shard_map(_body, mesh=mesh, in_specs=in_specs,
                           out_specs=out_specs, check_rep=False),
                 keep_unused=True)
    return {"fn": fn, "in_names": in_names, "out_names": out_names,
            "out_shapes": out_shapes, "nc": nc}


def _run(ex, ins):
    """ins: dict name -> [NCORES, *shape]; returns dict name -> [NCORES, *shape]."""
    concat_in = [np.ascontiguousarray(ins[n].reshape(-1, *ins[n].shape[2:]))
                 for n in ex["in_names"]]
    concat_zeros = [np.zeros((NCORES * s[0], *s[1:]), dt)
                    for (s, dt) in ex["out_shapes"]]
    out_arrs = ex["fn"](*concat_in, *concat_zeros)
    return {name: np.asarray(out_arrs[i]).reshape(NCORES, *ex["out_shapes"][i][0])
            for i, name in enumerate(ex["out_names"])}


def kernel(desc0, desc1):
    global _cached
    desc0 = np.asarray(desc0, dtype=np.float32)
    desc1 = np.asarray(desc1, dtype=np.float32)
    assert desc0.shape == (B, N, D) and desc1.shape == (B, M, D)

    if _cached is None:
        _cached = (_make_exec(_build1()), _make_exec(_build2()))
    ex1, ex2 = _cached

    import ml_dtypes
    bf16 = ml_dtypes.bfloat16
    a_slab = np.stack([desc0[b, h * HALF:(h + 1) * HALF]
                       for b in range(B) for h in range(2)])      # [8,4096,128]
    bt_all = np.stack([desc1[b].transpose(1, 0)
                       for b in range(B) for h in range(2)])      # [8,128,8192]
    at_all = a_slab.transpose(0, 2, 1)                            # [8,128,4096]

    r1 = _run(ex1, {"at": at_all.astype(bf16), "bt": bt_all.astype(bf16)})

    # host glue: score/chunk-argmax + column reduce + grouping for phase 2
    cm = r1["cm"].reshape(NCORES, 128, NT, NCHUNK).transpose(0, 2, 1, 3) \
                 .reshape(NCORES, HALF, NCHUNK)
    cm_b32 = cm.reshape(B, N, NCHUNK).astype(np.float32)  # f32: fast numpy
    score0_h = cm_b32.max(axis=2).astype(np.float16)              # [B,N] fp16
    cstar = cm_b32.argmax(axis=2)                                 # [B, N]
    colmax = np.empty((B, M), np.float16)
    colmax[:, :CD] = r1["colacc"].astype(np.float32) \
        .reshape(B, 2 * 128, CD).max(axis=1).astype(np.float16)
    colmax[:, CD:] = r1["colp"].astype(np.float32) \
        .reshape(B, 2 * NT, M - CD).max(axis=1).astype(np.float16)

    at2 = np.zeros((NCORES, D, NROWS2), np.float32)
    bt2_all = np.zeros((NCORES, 128, GRP * CW), np.float32)
    sg = np.full((NCORES, 128, NST), np.inf, np.float16)
    slot_of_row = np.full((B, N), -1, np.int64)
    core_of_row = np.full((B, N), 0, np.int64)
    overflow = []                                                 # (b, n)
    for b in range(B):
        d1t = desc1[b].T                                          # [128, M]
        for h2 in range(2):
            core = 2 * b + h2
            gchunks = range(h2 * GRP, (h2 + 1) * GRP)
            rows_of = {g: np.nonzero(cstar[b] == g)[0] for g in gchunks}
            # biggest groups get the big slots
            order = sorted(gchunks, key=lambda g: -len(rows_of[g]))
            for slot, g in enumerate(order):
                rws = rows_of[g]
                if len(rws) > SLOTS[slot]:
                    overflow.extend((b, n) for n in rws[SLOTS[slot]:])
                    rws = rws[:SLOTS[slot]]
                slots = SLOT_BASE[slot] + np.arange(len(rws))
                slot_of_row[b, rws] = slots
                core_of_row[b, rws] = core
                at2[core][:, slots] = desc0[b, rws].T
                sg[core][slots % 128, slots // 128] = score0_h[b, rws]
                bt2_all[core][:, slot * CW:(slot + 1) * CW] = \
                    d1t[:, g * CW:(g + 1) * CW]

    sg8 = np.repeat(sg, 8, axis=2)                         # [8,128,NST*8]
    r2 = _run(ex2, {"at2": at2.astype(bf16), "bt2": bt2_all.astype(bf16),
                    "sg": sg8})
    within = r2["idx"][:, :, ::8]                                 # [8, 128, NST]

    sl = np.maximum(slot_of_row, 0)
    cr = core_of_row
    w = within[cr, sl % 128, sl // 128].astype(np.int64)          # [B, N]
    np.clip(w, 0, CW - 1, out=w)
    match01 = (cstar * CW + w).astype(np.int32)
    score0 = score0_h.astype(np.float32)
    valid = (score0 > 0.1) & \
            (score0_h == np.take_along_axis(colmax, match01.astype(np.int64),
                                            axis=1))

    for b, n in overflow:                                         # ~never taken
        simrow = desc0[b, n] @ desc1[b].T
        j = int(simrow.argmax())
        s = simrow.max()
        col = desc0[b] @ desc1[b, j]
        match01[b, n] = j
        score0[b, n] = np.float32(s)
        valid[b, n] = (s > 0.1) & (int(col.argmax()) == n)

    return match01, score0, valid


# revision 81
# speedup vs baseline: 1.0024x; 1.0024x over previous
"""Trainium2 Bass kernel for DescriptorMatcher (mutual nearest neighbor matching).

Problem: given desc0 [B,N,D], desc1 [B,M,D] (B=4, N=M=8192, D=128, fp32):
    sim     = desc0 @ desc1^T                      [B,N,M]
    score0  = max_m sim                            [B,N]
    match01 = argmax_m sim                         [B,N]
    match10 = argmax_n sim                         [B,M]
    valid   = (match10[match01[n]] == n) & (score0 > 0.1)
returns (match01, score0, valid).

Key reformulation: the mutual check never needs match10 indices:
    match10[match01[n]] == n  <=>  score0[n] == colmax[match01[n]]
when all maxima are taken over the SAME rounded values. All on-device
max bookkeeping runs on fp16-rounded copies of the PSUM values: fp16
rounding is monotonic, so max(fp16(v)) == fp16(max(v)) and the
equality trick holds bit-exactly in the fp16 value system. Matmul
inputs ship as bf16 (halves the bandwidth-bound input DMA; the PE
accumulates bf16 products in fp32). Measured argmax flips vs the fp32
reference (bf16 inputs + fp16 value pipeline): 4.8e-3 (gate 2e-2).

Engine constraints that shape the design (walrus BIR verifier): GPSIMD
(Pool) cannot access PSUM and has no tensor_tensor/tensor_scalar ucode
(only partition_all_reduce/memset/ISA lib); TRN2 matmul output must be
fp32; ACT accum is sum-only; DVE fp16 runs tensor_scalar at 4x and
tensor_tensor at 2x (SBUF operands).

Phase 1 (per core = batch x row-half), per 128-row tile [128 x 8192]:
    PE:   16 fp32r matmuls; chunks 0-11 into [128,1536] PSUM tiles
          (bufs=2), chunks 12-15 into [128,512] PSUM tiles (bufs=2)
    ACT:  4x 1536-wide + 1x 512-wide PSUM->SBUF fp16 copies (chunks
          0-12) — ACT is the bottleneck engine at ~100% busy
    DVE:  fused TS copy+chunk-max chunks 13-15 (PSUM src), 13 junk-TS
          chunk-maxes on fp16 (4x mode), colacc[0:CD] = max(.., row)
          (emitted one tile late so pieces are always ready)
    Pool: per-tile partition_all_reduce of row[CD:M] (2 pieces),
          DMA'd out per tile; host folds the 32 per-tile partials
Outputs cm [128, 32*16] fp16 chunk maxima, colacc [128, CD] fp16 and
colp [32, M-CD]; the final column reduces happen on the host.

Phase 2: rows grouped by winning chunk (host); recompute sim[:, chunk]
with identically-laid-out bf16 matmuls -> bit-identical PSUM -> ACT
fp16 copy -> max_index(cm_value, chunk_fp16) = exact first-occurrence
within-chunk argmax. match01 = chunk*512 + within. 36 subtiles with
slot capacities (640x4, 512x4); the host assigns the largest chunk-
groups to the big slots by permuting bt2/at2 slot contents.

Rows overflowing a slot's capacity fall back to a host recompute.
"""

import numpy as np

import concourse.bass as bass  # noqa: F401  (bass must import before tile)
import concourse.mybir as mybir
import concourse.tile as tile
from concourse import bacc, bass_isa

B, N, M, D = 4, 8192, 8192, 128
NCORES = 8
HALF = N // 2          # rows per phase-1 core
NT = HALF // 128       # 32 n-tiles per core
CW = 512               # chunk width (phase-2 recompute width)
NCHUNK = M // CW       # 16 chunks per row
CD = 3712              # colacc columns on DVE; Pool reduces [CD, M) per tile
GRP = NCHUNK // 2      # 8 chunk-groups per phase-2 core
# Per-core slot capacities (rows) for the 8 chunk-groups: the host assigns
# the largest groups to the big slots (counts are ~N(512, 22), max observed
# 580 on the reference inputs); rows beyond a slot's capacity fall back to
# a host-side recompute.
SLOTS = (640, 640, 640, 640, 512, 512, 512, 512)
SLOT_BASE = tuple(int(x) for x in np.cumsum((0,) + SLOTS)[:-1])
NROWS2 = sum(SLOTS)    # 4608 phase-2 row slots per core
NST = NROWS2 // 128    # 36 phase-2 sub-tiles


def _build1():
    f32 = mybir.dt.float32
    f32r = mybir.dt.float32r
    f16 = mybir.dt.float16
    nc = bacc.Bacc("TRN2", target_bir_lowering=False, debug=False,
                   num_devices=NCORES)
    bf16 = mybir.dt.bfloat16
    at = nc.dram_tensor("at", [D, HALF], bf16, kind="ExternalInput").ap()
    bt = nc.dram_tensor("bt", [D, M], bf16, kind="ExternalInput").ap()
    cm_o = nc.dram_tensor("cm", [128, NT * NCHUNK], f16,
                          kind="ExternalOutput").ap()
    colacc_o = nc.dram_tensor("colacc", [128, CD], f16,
                              kind="ExternalOutput").ap()
    colp_o = nc.dram_tensor("colp", [NT, M - CD], f16,
                            kind="ExternalOutput").ap()

    with tile.TileContext(nc) as tc:
        with tc.tile_pool(name="big", bufs=1) as big, \
             tc.tile_pool(name="rows", bufs=5) as rows, \
             tc.tile_pool(name="dmy", bufs=8) as dmy, \
             tc.tile_pool(name="cps", bufs=3) as cps, \
             tc.tile_pool(name="psa", bufs=2, space="PSUM") as psa, \
             tc.tile_pool(name="psp", bufs=2, space="PSUM") as psp:
            atb = big.tile([128, HALF], bf16, name="atb")
            btb = big.tile([128, M], bf16, name="btb")
            # tile 0 needs at[:, 0:128] and then bt chunks in matmul order;
            # front-load tiny slices on BOTH DGE queues so the PE starts ASAP
            nc.scalar.dma_start(atb[:, 0:128], at[:, 0:128])
            nc.sync.dma_start(btb[:, 0:512], bt[:, 0:512])
            nc.sync.dma_start(btb[:, 512:1024], bt[:, 512:1024])
            nc.scalar.dma_start(btb[:, 1024:2048], bt[:, 1024:2048])
            # rest of bt in wide transfers so tile 0's tail chunks aren't
            # starved behind a long descriptor queue; at after bt.
            for c in range(2048, M, 3072):
                w = min(3072, M - c)
                nc.sync.dma_start(btb[:, c:c + w], bt[:, c:c + w])
            nc.sync.dma_start(atb[:, 128:1024], at[:, 128:1024])
            for c in range(1024, HALF, 3072):
                w = min(3072, HALF - c)
                nc.sync.dma_start(atb[:, c:c + w], at[:, c:c + w])
            cm_all = big.tile([128, NT * NCHUNK], f16, name="cm_all")
            colacc = big.tile([128, M], f16, name="colacc")

            # colacc piece boundaries (moderate quanta: overhead vs blocking)
            dve_cuts = [0, CD]

            def colacc_update(tp, prow):
                """Column-max bookkeeping for row(tp). DVE accumulates
                colacc = max(colacc, row) on [0:CD]; Pool (which cannot
                read PSUM and has no tensor_tensor ucode) instead does a
                per-tile 128-partition max-reduce of row[CD:M], DMA'd out
                for the host to fold over tiles. Emitted one tile LATE so
                the pieces are ready when the engine queues reach them."""
                cuts = dve_cuts
                for lo, hi in zip(cuts[:-1], cuts[1:]):
                    if tp == 0:
                        nc.vector.tensor_copy(colacc[:, lo:hi], prow[:, lo:hi])
                    else:
                        nc.vector.tensor_tensor(colacc[:, lo:hi],
                                                colacc[:, lo:hi],
                                                prow[:, lo:hi],
                                                op=mybir.AluOpType.max)
                    if tp == NT - 1:
                        nc.sync.dma_start(colacc_o[:, lo:hi],
                                          colacc[:, lo:hi])
                # two pieces: the first needs only earlier chunks, so it
                # starts before the tail chunks land (shortens the drain)
                cp = cps.tile([128, M - CD], f16, tag="cp", name="cp")
                pw = (M - CD) // 2
                for q in range(2):
                    lo = CD + q * pw
                    hi = M if q == 1 else lo + pw
                    nc.gpsimd.partition_all_reduce(
                        cp[:, lo - CD:hi - CD], prow[:, lo:hi], channels=128,
                        reduce_op=bass_isa.ReduceOp.max)
                nc.sync.dma_start(colp_o[tp:tp + 1, :], cp[0:1, :])

            def emit_pa(t, h, row, cmt):
                pa = psa.tile([128, 1536], f32, tag="pa", name="pa")
                for j in range(3):
                    mlo = h * 1536 + j * 512
                    nc.tensor.matmul(pa[:, j * 512:(j + 1) * 512],
                                     atb[:, t * 128:(t + 1) * 128],
                                     btb[:, mlo:mlo + 512],
                                     start=True, stop=True)
                # PSUM escape: fp32 -> fp16 on ACT
                nc.scalar.copy(row[:, h * 1536:(h + 1) * 1536], pa[:])
                # chunk maxima for ACT-copied chunks (DVE 4x mode);
                # junk output so colacc only depends on copies.
                dj = dmy.tile([128, 1536], f16, tag="dmy", name="dmy")
                for j in range(3):
                    ch = h * 3 + j
                    nc.vector.tensor_scalar(
                        dj[:, j * CW:(j + 1) * CW],
                        row[:, ch * CW:(ch + 1) * CW], 1.0, None,
                        op0=mybir.AluOpType.mult,
                        op1=mybir.AluOpType.max,
                        accum_out=cmt[:, ch:ch + 1])

            def emit_psp(t, row, cmt):
                for ch in range(12, 16):
                    pp = psp.tile([128, CW], f32, tag="pp", name="pp")
                    nc.tensor.matmul(pp[:],
                                     atb[:, t * 128:(t + 1) * 128],
                                     btb[:, ch * CW:(ch + 1) * CW],
                                     start=True, stop=True)
                    if ch == 12:
                        # chunk 12: ACT copy + DVE junk-TS max
                        nc.scalar.copy(row[:, 12 * CW:13 * CW], pp[:])
                        dj = dmy.tile([128, 1024], f16, tag="dmy", name="dmy")
                        nc.vector.tensor_scalar(
                            dj[:, 0:CW], row[:, 12 * CW:13 * CW], 1.0, None,
                            op0=mybir.AluOpType.mult,
                            op1=mybir.AluOpType.max,
                            accum_out=cmt[:, 12:13])
                    else:
                        # chunks 13-15: DVE-fused copy+chunk-max (PSUM src)
                        nc.vector.tensor_scalar(
                            row[:, ch * CW:(ch + 1) * CW], pp[:],
                            1.0, None,
                            op0=mybir.AluOpType.mult,
                            op1=mybir.AluOpType.max,
                            accum_out=cmt[:, ch:ch + 1])

            prev_row = None
            for t in range(NT):
                row = rows.tile([128, M], f16, tag="row", name="row")
                cmt = cm_all[:, t * NCHUNK:(t + 1) * NCHUNK]
                # Chunks 0-11: PE fills [128,1024] PSUM tiles (bufs=3, so
                # PE runs ahead of the ACT escape copies). GPSIMD cannot
                # read PSUM, so the remaining chunks 12-15 go through psp
                # tiles: chunk 12 ACT-copied, 13-15 DVE-fused copy+max.
                # Last tile: tail chunks first so the final colacc and
                # partition-reduce pieces overlap the last ACT copies.
                if t == NT - 1:
                    emit_psp(t, row, cmt)
                for h in range(4):
                    emit_pa(t, h, row, cmt)
                if t != NT - 1:
                    emit_psp(t, row, cmt)
                # delayed colacc update for the previous tile
                if prev_row is not None:
                    colacc_update(t - 1, prev_row)
                prev_row = row
                # stagger the (tiny) chunk-maxima output
                if t % 8 == 7 and t != NT - 1:
                    nc.sync.dma_start(
                        cm_o[:, (t - 7) * NCHUNK:(t + 1) * NCHUNK],
                        cm_all[:, (t - 7) * NCHUNK:(t + 1) * NCHUNK])
            colacc_update(NT - 1, prev_row)
            nc.sync.dma_start(cm_o[:, (NT - 8) * NCHUNK:],
                              cm_all[:, (NT - 8) * NCHUNK:])
    nc.compile()
    return nc


def _build2():
    f32, f32r, u32 = mybir.dt.float32, mybir.dt.float32r, mybir.dt.uint32
    f16 = mybir.dt.float16
    nc = bacc.Bacc("TRN2", target_bir_lowering=False, debug=False,
                   num_devices=NCORES)
    bf16 = mybir.dt.bfloat16
    at2 = nc.dram_tensor("at2", [D, NROWS2], bf16, kind="ExternalInput").ap()
    bt2 = nc.dram_tensor("bt2", [D, M // 2], bf16, kind="ExternalInput").ap()
    sg = nc.dram_tensor("sg", [128, NST * 8], f16, kind="ExternalInput").ap()
    idx_o = nc.dram_tensor("idx", [128, NST * 8], u32, kind="ExternalOutput").ap()
    with tile.TileContext(nc) as tc:
        with tc.tile_pool(name="big", bufs=1) as big, \
             tc.tile_pool(name="stg", bufs=4) as stg, \
             tc.tile_pool(name="ps", bufs=4, space="PSUM") as ps:
            a2b = big.tile([128, NROWS2], bf16, name="a2b")
            b2b = big.tile([128, M // 2], bf16, name="b2b")
            sgb = big.tile([128, NST * 8], f16, name="sgb")
            # first slices on both DGE queues in parallel
            nc.scalar.dma_start(a2b[:, 0:128], at2[:, 0:128])
            nc.sync.dma_start(b2b[:, 0:512], bt2[:, 0:512])
            nc.scalar.dma_start(sgb[:], sg[:])
            # interleave so group 0's matmuls start before all input lands
            na = (NROWS2 + 1023) // 1024
            nb = (M // 2) // 1024
            for i in range(max(na, nb)):
                if i < na:
                    c = i * 1024
                    lo = 128 if i == 0 else 0
                    w = min(1024, NROWS2 - c)
                    nc.sync.dma_start(a2b[:, c + lo:c + w],
                                      at2[:, c + lo:c + w])
                if i < nb:
                    c = i * 1024
                    lo = 512 if i == 0 else 0
                    nc.sync.dma_start(b2b[:, c + lo:c + 1024],
                                      bt2[:, c + lo:c + 1024])
            idx8 = big.tile([128, NST * 8], u32, name="idx8")
            st = -1
            for g in range(GRP):
                for k in range(SLOTS[g] // 128):
                    st += 1
                    pt = ps.tile([128, CW], f32, tag="pt", name="pt")
                    nc.tensor.matmul(pt[:],
                                     a2b[:, st * 128:(st + 1) * 128],
                                     b2b[:, g * CW:(g + 1) * CW],
                                     start=True, stop=True)
                    ch = stg.tile([128, CW], f16, tag="ch", name="ch")
                    nc.scalar.copy(ch[:], pt[:])
                    nc.vector.max_index(idx8[:, st * 8:(st + 1) * 8],
                                        sgb[:, st * 8:(st + 1) * 8], ch[:])
            nc.sync.dma_start(idx_o[:, 0:(NST - 1) * 8],
                              idx8[:, 0:(NST - 1) * 8])
            nc.sync.dma_start(idx_o[:, (NST - 1) * 8:],
                              idx8[:, (NST - 1) * 8:])
    nc.compile()
    return nc


_cached = None


def _make_exec(nc):
    import jax
    from jax.sharding import Mesh, PartitionSpec
    from jax.experimental.shard_map import shard_map
    from concourse import bass2jax
    from concourse.bass2jax import _bass_exec_p

    partition_name = nc.partition_id_tensor.name if nc.partition_id_tensor else None
    in_names, out_names, out_avals, out_shapes = [], [], [], []
    for alloc in nc.m.functions[0].allocations:
        if not isinstance(alloc, mybir.MemoryLocationSet):
            continue
        name = alloc.memorylocations[0].name
        if alloc.kind == "ExternalInput":
            if name != partition_name:
                in_names.append(name)
        elif alloc.kind == "ExternalOutput":
            shape = tuple(alloc.tensor_shape)
            dtype = mybir.dt.np(alloc.dtype)
            out_names.append(name)
            out_shapes.append((shape, dtype))
            out_avals.append(jax.core.ShapedArray(shape, dtype))
    n_params = len(in_names)
    n_outs = len(out_names)
    all_in_names = in_names + out_names
    if partition_name is not None:
        all_in_names = all_in_names + [partition_name]

    def _body(*args):
        operands = list(args)
        if partition_name is not None:
            operands.append(bass2jax.partition_id_tensor())
        outs = _bass_exec_p.bind(
            *operands, out_avals=tuple(out_avals), in_names=tuple(all_in_names),
            out_names=tuple(out_names), lowering_input_output_aliases=(),
            sim_require_finite=True, sim_require_nnan=True, nc=nc)
        return tuple(outs)

    devices = jax.devices()[:NCORES]
    mesh = Mesh(np.asarray(devices), ("core",))
    in_specs = (PartitionSpec("core"),) * (n_params + n_outs)
    out_specs = (PartitionSpec("core"),) * n_outs
    fn = jax.jit(shard_map(_body, mesh=mesh, in_specs=in_specs,
                           out_specs=out_specs, check_rep=False),
                 keep_unused=True)
    return {"fn": fn, "in_names": in_names, "out_names": out_names,
            "out_shapes": out_shapes, "nc": nc}


def _run(ex, ins):
    """ins: dict name -> [NCORES, *shape]; returns dict name -> [NCORES, *shape]."""
    concat_in = [np.ascontiguousarray(ins[n].reshape(-1, *ins[n].shape[2:]))
                 for n in ex["in_names"]]
    concat_zeros = [np.zeros((NCORES * s[0], *s[1:]), dt)
                    for (s, dt) in ex["out_shapes"]]
    out_arrs = ex["fn"](*concat_in, *concat_zeros)
    return {name: np.asarray(out_arrs[i]).reshape(NCORES, *ex["out_shapes"][i][0])
            for i, name in enumerate(ex["out_names"])}


def kernel(desc0, desc1):
    global _cached
    desc0 = np.asarray(desc0, dtype=np.float32)
    desc1 = np.asarray(desc1, dtype=np.float32)
    assert desc0.shape == (B, N, D) and desc1.shape == (B, M, D)

    if _cached is None:
        _cached = (_make_exec(_build1()), _make_exec(_build2()))
    ex1, ex2 = _cached

    import ml_dtypes
    bf16 = ml_dtypes.bfloat16
    a_slab = np.stack([desc0[b, h * HALF:(h + 1) * HALF]
                       for b in range(B) for h in range(2)])      # [8,4096,128]
    bt_all = np.stack([desc1[b].transpose(1, 0)
                       for b in range(B) for h in range(2)])      # [8,128,8192]
    at_all = a_slab.transpose(0, 2, 1)                            # [8,128,4096]

    r1 = _run(ex1, {"at": at_all.astype(bf16), "bt": bt_all.astype(bf16)})

    # host glue: score/chunk-argmax + column reduce + grouping for phase 2
    cm = r1["cm"].reshape(NCORES, 128, NT, NCHUNK).transpose(0, 2, 1, 3) \
                 .reshape(NCORES, HALF, NCHUNK)
    cm_b32 = cm.reshape(B, N, NCHUNK).astype(np.float32)  # f32: fast numpy
    score0_h = cm_b32.max(axis=2).astype(np.float16)              # [B,N] fp16
    cstar = cm_b32.argmax(axis=2)                                 # [B, N]
    colmax = np.empty((B, M), np.float16)
    colmax[:, :CD] = r1["colacc"].astype(np.float32) \
        .reshape(B, 2 * 128, CD).max(axis=1).astype(np.float16)
    colmax[:, CD:] = r1["colp"].astype(np.float32) \
        .reshape(B, 2 * NT, M - CD).max(axis=1).astype(np.float16)

    at2 = np.zeros((NCORES, D, NROWS2), np.float32)
    bt2_all = np.zeros((NCORES, 128, GRP * CW), np.float32)
    sg = np.full((NCORES, 128, NST), np.inf, np.float16)
    slot_of_row = np.full((B, N), -1, np.int64)
    core_of_row = np.full((B, N), 0, np.int64)
    overflow = []                                                 # (b, n)
    for b in range(B):
        d1t = desc1[b].T                                          # [128, M]
        for h2 in range(2):
            core = 2 * b + h2
            gchunks = range(h2 * GRP, (h2 + 1) * GRP)
            rows_of = {g: np.nonzero(cstar[b] == g)[0] for g in gchunks}
            # biggest groups get the big slots
            order = sorted(gchunks, key=lambda g: -len(rows_of[g]))
            for slot, g in enumerate(order):
                rws = rows_of[g]
                if len(rws) > SLOTS[slot]:
                    overflow.extend((b, n) for n in rws[SLOTS[slot]:])
                    rws = rws[:SLOTS[slot]]
                slots = SLOT_BASE[slot] + np.arange(len(rws))
                slot_of_row[b, rws] = slots
                core_of_row[b, rws] = core
                at2[core][:, slots] = desc0[b, rws].T
                sg[core][slots % 128, slots // 128] = score0_h[b, rws]
                bt2_all[core][:, slot * CW:(slot + 1) * CW] = \
                    d1t[:, g * CW:(g + 1) * CW]

    sg8 = np.repeat(sg, 8, axis=2)                         # [8,128,NST*8]
    r2 = _run(ex2, {"at2": at2.astype(bf16), "bt2": bt2_all.astype(bf16),
                    "sg": sg8})
    within = r2["idx"][:, :, ::8]                                 # [8, 128, NST]

    sl = np.maximum(slot_of_row, 0)
    cr = core_of_row
    w = within[cr, sl % 128, sl // 128].astype(np.int64)          # [B, N]
    np.clip(w, 0, CW - 1, out=w)
    match01 = (cstar * CW + w).astype(np.int32)
    score0 = score0_h.astype(np.float32)
    valid = (score0 > 0.1) & \
            (score0_h == np.take_along_axis(colmax, match01.astype(np.int64),
                                            axis=1))

    for b, n in overflow:                                         # ~never taken
        simrow = desc0[b, n] @ desc1[b].T
        j = int(simrow.argmax())
        s = simrow.max()
        col = desc0[b] @ desc1[b, j]
        match01[b, n] = j
        score0[b, n] = np.float32(s)
        valid[b, n] = (s > 0.1) & (int(col.argmax()) == n)

    return match01, score0, valid


# revision 82
# speedup vs baseline: 1.0048x; 1.0024x over previous
"""Trainium2 Bass kernel for DescriptorMatcher (mutual nearest neighbor matching).

Problem: given desc0 [B,N,D], desc1 [B,M,D] (B=4, N=M=8192, D=128, fp32):
    sim     = desc0 @ desc1^T                      [B,N,M]
    score0  = max_m sim                            [B,N]
    match01 = argmax_m sim                         [B,N]
    match10 = argmax_n sim                         [B,M]
    valid   = (match10[match01[n]] == n) & (score0 > 0.1)
returns (match01, score0, valid).

Key reformulation: the mutual check never needs match10 indices:
    match10[match01[n]] == n  <=>  score0[n] == colmax[match01[n]]
when all maxima are taken over the SAME rounded values. All on-device
max bookkeeping runs on fp16-rounded copies of the PSUM values: fp16
rounding is monotonic, so max(fp16(v)) == fp16(max(v)) and the
equality trick holds bit-exactly in the fp16 value system. Matmul
inputs ship as bf16 (halves the bandwidth-bound input DMA; the PE
accumulates bf16 products in fp32). Measured argmax flips vs the fp32
reference (bf16 inputs + fp16 value pipeline): 4.8e-3 (gate 2e-2).

Engine constraints that shape the design (walrus BIR verifier): GPSIMD
(Pool) cannot access PSUM and has no tensor_tensor/tensor_scalar ucode
(only partition_all_reduce/memset/ISA lib); TRN2 matmul output must be
fp32; ACT accum is sum-only; DVE fp16 runs tensor_scalar at 4x and
tensor_tensor at 2x (SBUF operands).

Phase 1 (per core = batch x row-half), per 128-row tile [128 x 8192]:
    PE:   16 fp32r matmuls; chunks 0-11 into [128,1536] PSUM tiles
          (bufs=2), chunks 12-15 into [128,512] PSUM tiles (bufs=2)
    ACT:  4x 1536-wide + 1x 512-wide PSUM->SBUF fp16 copies (chunks
          0-12) — ACT is the bottleneck engine at ~100% busy
    DVE:  fused TS copy+chunk-max chunks 13-15 (PSUM src), 13 junk-TS
          chunk-maxes on fp16 (4x mode), colacc[0:CD] = max(.., row)
          (emitted one tile late so pieces are always ready)
    Pool: per-tile partition_all_reduce of row[CD:M] (2 pieces),
          DMA'd out per tile; host folds the 32 per-tile partials
Outputs cm [128, 32*16] fp16 chunk maxima, colacc [128, CD] fp16 and
colp [32, M-CD]; the final column reduces happen on the host.

Phase 2: rows grouped by winning chunk (host); recompute sim[:, chunk]
with identically-laid-out bf16 matmuls -> bit-identical PSUM -> ACT
fp16 copy -> max_index(cm_value, chunk_fp16) = exact first-occurrence
within-chunk argmax. match01 = chunk*512 + within. 36 subtiles with
slot capacities (640x4, 512x4); the host assigns the largest chunk-
groups to the big slots by permuting bt2/at2 slot contents.

Rows overflowing a slot's capacity fall back to a host recompute.
"""

import numpy as np

import concourse.bass as bass  # noqa: F401  (bass must import before tile)
import concourse.mybir as mybir
import concourse.tile as tile
from concourse import bacc, bass_isa

B, N, M, D = 4, 8192, 8192, 128
NCORES = 8
HALF = N // 2          # rows per phase-1 core
NT = HALF // 128       # 32 n-tiles per core
CW = 512               # chunk width (phase-2 recompute width)
NCHUNK = M // CW       # 16 chunks per row
CD = 3712              # colacc columns on DVE; Pool reduces [CD, M) per tile
GRP = NCHUNK // 2      # 8 chunk-groups per phase-2 core
# Per-core slot capacities (rows) for the 8 chunk-groups: the host assigns
# the largest groups to the big slots (counts are ~N(512, 22), max observed
# 580 on the reference inputs); rows beyond a slot's capacity fall back to
# a host-side recompute.
SLOTS = (640, 640, 640, 512, 512, 512, 512, 512)
SLOT_BASE = tuple(int(x) for x in np.cumsum((0,) + SLOTS)[:-1])
NROWS2 = sum(SLOTS)    # 4608 phase-2 row slots per core
NST = NROWS2 // 128    # 36 phase-2 sub-tiles


def _build1():
    f32 = mybir.dt.float32
    f32r = mybir.dt.float32r
    f16 = mybir.dt.float16
    nc = bacc.Bacc("TRN2", target_bir_lowering=False, debug=False,
                   num_devices=NCORES)
    bf16 = mybir.dt.bfloat16
    at = nc.dram_tensor("at", [D, HALF], bf16, kind="ExternalInput").ap()
    bt = nc.dram_tensor("bt", [D, M], bf16, kind="ExternalInput").ap()
    cm_o = nc.dram_tensor("cm", [128, NT * NCHUNK], f16,
                          kind="ExternalOutput").ap()
    colacc_o = nc.dram_tensor("colacc", [128, CD], f16,
                              kind="ExternalOutput").ap()
    colp_o = nc.dram_tensor("colp", [NT, M - CD], f16,
                            kind="ExternalOutput").ap()

    with tile.TileContext(nc) as tc:
        with tc.tile_pool(name="big", bufs=1) as big, \
             tc.tile_pool(name="rows", bufs=5) as rows, \
             tc.tile_pool(name="dmy", bufs=8) as dmy, \
             tc.tile_pool(name="cps", bufs=3) as cps, \
             tc.tile_pool(name="psa", bufs=2, space="PSUM") as psa, \
             tc.tile_pool(name="psp", bufs=2, space="PSUM") as psp:
            atb = big.tile([128, HALF], bf16, name="atb")
            btb = big.tile([128, M], bf16, name="btb")
            # tile 0 needs at[:, 0:128] and then bt chunks in matmul order;
            # front-load tiny slices on BOTH DGE queues so the PE starts ASAP
            nc.scalar.dma_start(atb[:, 0:128], at[:, 0:128])
            nc.sync.dma_start(btb[:, 0:512], bt[:, 0:512])
            nc.sync.dma_start(btb[:, 512:1024], bt[:, 512:1024])
            nc.scalar.dma_start(btb[:, 1024:2048], bt[:, 1024:2048])
            # rest of bt in wide transfers so tile 0's tail chunks aren't
            # starved behind a long descriptor queue; at after bt.
            for c in range(2048, M, 3072):
                w = min(3072, M - c)
                nc.sync.dma_start(btb[:, c:c + w], bt[:, c:c + w])
            nc.sync.dma_start(atb[:, 128:1024], at[:, 128:1024])
            for c in range(1024, HALF, 3072):
                w = min(3072, HALF - c)
                nc.sync.dma_start(atb[:, c:c + w], at[:, c:c + w])
            cm_all = big.tile([128, NT * NCHUNK], f16, name="cm_all")
            colacc = big.tile([128, M], f16, name="colacc")

            # colacc piece boundaries (moderate quanta: overhead vs blocking)
            dve_cuts = [0, CD]

            def colacc_update(tp, prow):
                """Column-max bookkeeping for row(tp). DVE accumulates
                colacc = max(colacc, row) on [0:CD]; Pool (which cannot
                read PSUM and has no tensor_tensor ucode) instead does a
                per-tile 128-partition max-reduce of row[CD:M], DMA'd out
                for the host to fold over tiles. Emitted one tile LATE so
                the pieces are ready when the engine queues reach them."""
                cuts = dve_cuts
                for lo, hi in zip(cuts[:-1], cuts[1:]):
                    if tp == 0:
                        nc.vector.tensor_copy(colacc[:, lo:hi], prow[:, lo:hi])
                    else:
                        nc.vector.tensor_tensor(colacc[:, lo:hi],
                                                colacc[:, lo:hi],
                                                prow[:, lo:hi],
                                                op=mybir.AluOpType.max)
                    if tp == NT - 1:
                        nc.sync.dma_start(colacc_o[:, lo:hi],
                                          colacc[:, lo:hi])
                # two pieces: the first needs only earlier chunks, so it
                # starts before the tail chunks land (shortens the drain)
                cp = cps.tile([128, M - CD], f16, tag="cp", name="cp")
                pw = (M - CD) // 2
                for q in range(2):
                    lo = CD + q * pw
                    hi = M if q == 1 else lo + pw
                    nc.gpsimd.partition_all_reduce(
                        cp[:, lo - CD:hi - CD], prow[:, lo:hi], channels=128,
                        reduce_op=bass_isa.ReduceOp.max)
                nc.sync.dma_start(colp_o[tp:tp + 1, :], cp[0:1, :])

            def emit_pa(t, h, row, cmt):
                pa = psa.tile([128, 1536], f32, tag="pa", name="pa")
                for j in range(3):
                    mlo = h * 1536 + j * 512
                    nc.tensor.matmul(pa[:, j * 512:(j + 1) * 512],
                                     atb[:, t * 128:(t + 1) * 128],
                                     btb[:, mlo:mlo + 512],
                                     start=True, stop=True)
                # PSUM escape: fp32 -> fp16 on ACT
                nc.scalar.copy(row[:, h * 1536:(h + 1) * 1536], pa[:])
                # chunk maxima for ACT-copied chunks (DVE 4x mode);
                # junk output so colacc only depends on copies.
                dj = dmy.tile([128, 1536], f16, tag="dmy", name="dmy")
                for j in range(3):
                    ch = h * 3 + j
                    nc.vector.tensor_scalar(
                        dj[:, j * CW:(j + 1) * CW],
                        row[:, ch * CW:(ch + 1) * CW], 1.0, None,
                        op0=mybir.AluOpType.mult,
                        op1=mybir.AluOpType.max,
                        accum_out=cmt[:, ch:ch + 1])

            def emit_psp(t, row, cmt):
                for ch in range(12, 16):
                    pp = psp.tile([128, CW], f32, tag="pp", name="pp")
                    nc.tensor.matmul(pp[:],
                                     atb[:, t * 128:(t + 1) * 128],
                                     btb[:, ch * CW:(ch + 1) * CW],
                                     start=True, stop=True)
                    if ch == 12:
                        # chunk 12: ACT copy + DVE junk-TS max
                        nc.scalar.copy(row[:, 12 * CW:13 * CW], pp[:])
                        dj = dmy.tile([128, 1024], f16, tag="dmy", name="dmy")
                        nc.vector.tensor_scalar(
                            dj[:, 0:CW], row[:, 12 * CW:13 * CW], 1.0, None,
                            op0=mybir.AluOpType.mult,
                            op1=mybir.AluOpType.max,
                            accum_out=cmt[:, 12:13])
                    else:
                        # chunks 13-15: DVE-fused copy+chunk-max (PSUM src)
                        nc.vector.tensor_scalar(
                            row[:, ch * CW:(ch + 1) * CW], pp[:],
                            1.0, None,
                            op0=mybir.AluOpType.mult,
                            op1=mybir.AluOpType.max,
                            accum_out=cmt[:, ch:ch + 1])

            prev_row = None
            for t in range(NT):
                row = rows.tile([128, M], f16, tag="row", name="row")
                cmt = cm_all[:, t * NCHUNK:(t + 1) * NCHUNK]
                # Chunks 0-11: PE fills [128,1024] PSUM tiles (bufs=3, so
                # PE runs ahead of the ACT escape copies). GPSIMD cannot
                # read PSUM, so the remaining chunks 12-15 go through psp
                # tiles: chunk 12 ACT-copied, 13-15 DVE-fused copy+max.
                # Last tile: tail chunks first so the final colacc and
                # partition-reduce pieces overlap the last ACT copies.
                if t == NT - 1:
                    emit_psp(t, row, cmt)
                for h in range(4):
                    emit_pa(t, h, row, cmt)
                if t != NT - 1:
                    emit_psp(t, row, cmt)
                # delayed colacc update for the previous tile
                if prev_row is not None:
                    colacc_update(t - 1, prev_row)
                prev_row = row
                # stagger the (tiny) chunk-maxima output
                if t % 8 == 7 and t != NT - 1:
                    nc.sync.dma_start(
                        cm_o[:, (t - 7) * NCHUNK:(t + 1) * NCHUNK],
                        cm_all[:, (t - 7) * NCHUNK:(t + 1) * NCHUNK])
            colacc_update(NT - 1, prev_row)
            nc.sync.dma_start(cm_o[:, (NT - 8) * NCHUNK:],
                              cm_all[:, (NT - 8) * NCHUNK:])
    nc.compile()
    return nc


def _build2():
    f32, f32r, u32 = mybir.dt.float32, mybir.dt.float32r, mybir.dt.uint32
    f16 = mybir.dt.float16
    nc = bacc.Bacc("TRN2", target_bir_lowering=False, debug=False,
                   num_devices=NCORES)
    bf16 = mybir.dt.bfloat16
    at2 = nc.dram_tensor("at2", [D, NROWS2], bf16, kind="ExternalInput").ap()
    bt2 = nc.dram_tensor("bt2", [D, M // 2], bf16, kind="ExternalInput").ap()
    sg = nc.dram_tensor("sg", [128, NST * 8], f16, kind="ExternalInput").ap()
    idx_o = nc.dram_tensor("idx", [128, NST * 8], u32, kind="ExternalOutput").ap()
    with tile.TileContext(nc) as tc:
        with tc.tile_pool(name="big", bufs=1) as big, \
             tc.tile_pool(name="stg", bufs=4) as stg, \
             tc.tile_pool(name="ps", bufs=4, space="PSUM") as ps:
            a2b = big.tile([128, NROWS2], bf16, name="a2b")
            b2b = big.tile([128, M // 2], bf16, name="b2b")
            sgb = big.tile([128, NST * 8], f16, name="sgb")
            # first slices on both DGE queues in parallel
            nc.scalar.dma_start(a2b[:, 0:128], at2[:, 0:128])
            nc.sync.dma_start(b2b[:, 0:512], bt2[:, 0:512])
            nc.scalar.dma_start(sgb[:], sg[:])
            # interleave so group 0's matmuls start before all input lands
            na = (NROWS2 + 1023) // 1024
            nb = (M // 2) // 1024
            for i in range(max(na, nb)):
                if i < na:
                    c = i * 1024
                    lo = 128 if i == 0 else 0
                    w = min(1024, NROWS2 - c)
                    nc.sync.dma_start(a2b[:, c + lo:c + w],
                                      at2[:, c + lo:c + w])
                if i < nb:
                    c = i * 1024
                    lo = 512 if i == 0 else 0
                    nc.sync.dma_start(b2b[:, c + lo:c + 1024],
                                      bt2[:, c + lo:c + 1024])
            idx8 = big.tile([128, NST * 8], u32, name="idx8")
            st = -1
            for g in range(GRP):
                for k in range(SLOTS[g] // 128):
                    st += 1
                    pt = ps.tile([128, CW], f32, tag="pt", name="pt")
                    nc.tensor.matmul(pt[:],
                                     a2b[:, st * 128:(st + 1) * 128],
                                     b2b[:, g * CW:(g + 1) * CW],
                                     start=True, stop=True)
                    ch = stg.tile([128, CW], f16, tag="ch", name="ch")
                    nc.scalar.copy(ch[:], pt[:])
                    nc.vector.max_index(idx8[:, st * 8:(st + 1) * 8],
                                        sgb[:, st * 8:(st + 1) * 8], ch[:])
            nc.sync.dma_start(idx_o[:, 0:(NST - 1) * 8],
                              idx8[:, 0:(NST - 1) * 8])
            nc.sync.dma_start(idx_o[:, (NST - 1) * 8:],
                              idx8[:, (NST - 1) * 8:])
    nc.compile()
    return nc


_cached = None


def _make_exec(nc):
    import jax
    from jax.sharding import Mesh, PartitionSpec
    from jax.experimental.shard_map import shard_map
    from concourse import bass2jax
    from concourse.bass2jax import _bass_exec_p

    partition_name = nc.partition_id_tensor.name if nc.partition_id_tensor else None
    in_names, out_names, out_avals, out_shapes = [], [], [], []
    for alloc in nc.m.functions[0].allocations:
        if not isinstance(alloc, mybir.MemoryLocationSet):
            continue
        name = alloc.memorylocations[0].name
        if alloc.kind == "ExternalInput":
            if name != partition_name:
                in_names.append(name)
        elif alloc.kind == "ExternalOutput":
            shape = tuple(alloc.tensor_shape)
            dtype = mybir.dt.np(alloc.dtype)
            out_names.append(name)
            out_shapes.append((shape, dtype))
            out_avals.append(jax.core.ShapedArray(shape, dtype))
    n_params = len(in_names)
    n_outs = len(out_names)
    all_in_names = in_names + out_names
    if partition_name is not None:
        all_in_names = all_in_names + [partition_name]

    def _body(*args):
        operands = list(args)
        if partition_name is not None:
            operands.append(bass2jax.partition_id_tensor())
        outs = _bass_exec_p.bind(
            *operands, out_avals=tuple(out_avals), in_names=tuple(all_in_names),
            out_names=tuple(out_names), lowering_input_output_aliases=(),
            sim_require_finite=True, sim_require_nnan=True, nc=nc)
        return tuple(outs)

    devices = jax.devices()[:NCORES]
    mesh = Mesh(np.asarray(devices), ("core",))
    in_specs = (PartitionSpec("core"),) * (n_params + n_outs)
    out_specs = (PartitionSpec("core"),) * n_outs
    fn = jax.jit(shard_map(_body, mesh=mesh, in_specs=in_specs,
                           out_specs=out_specs, check_rep=False),
                 keep_unused=True)
    return {"fn": fn, "in_names": in_names, "out_names": out_names,
            "out_shapes": out_shapes, "nc": nc}


def _run(ex, ins):
    """ins: dict name -> [NCORES, *shape]; returns dict name -> [NCORES, *shape]."""
    concat_in = [np.ascontiguousarray(ins[n].reshape(-1, *ins[n].shape[2:]))
                 for n in ex["in_names"]]
    concat_zeros = [np.zeros((NCORES * s[0], *s[1:]), dt)
                    for (s, dt) in ex["out_shapes"]]
    out_arrs = ex["fn"](*concat_in, *concat_zeros)
    return {name: np.asarray(out_arrs[i]).reshape(NCORES, *ex["out_shapes"][i][0])
            for i, name in enumerate(ex["out_names"])}


def kernel(desc0, desc1):
    global _cached
    desc0 = np.asarray(desc0, dtype=np.float32)
    desc1 = np.asarray(desc1, dtype=np.float32)
    assert desc0.shape == (B, N, D) and desc1.shape == (B, M, D)

    if _cached is None:
        _cached = (_make_exec(_build1()), _make_exec(_build2()))
    ex1, ex2 = _cached

    import ml_dtypes
    bf16 = ml_dtypes.bfloat16
    a_slab = np.stack([desc0[b, h * HALF:(h + 1) * HALF]
                       for b in range(B) for h in range(2)])      # [8,4096,128]
    bt_all = np.stack([desc1[b].transpose(1, 0)
                       for b in range(B) for h in range(2)])      # [8,128,8192]
    at_all = a_slab.transpose(0, 2, 1)                            # [8,128,4096]

    r1 = _run(ex1, {"at": at_all.astype(bf16), "bt": bt_all.astype(bf16)})

    # host glue: score/chunk-argmax + column reduce + grouping for phase 2
    cm = r1["cm"].reshape(NCORES, 128, NT, NCHUNK).transpose(0, 2, 1, 3) \
                 .reshape(NCORES, HALF, NCHUNK)
    cm_b32 = cm.reshape(B, N, NCHUNK).astype(np.float32)  # f32: fast numpy
    score0_h = cm_b32.max(axis=2).astype(np.float16)              # [B,N] fp16
    cstar = cm_b32.argmax(axis=2)                                 # [B, N]
    colmax = np.empty((B, M), np.float16)
    colmax[:, :CD] = r1["colacc"].astype(np.float32) \
        .reshape(B, 2 * 128, CD).max(axis=1).astype(np.float16)
    colmax[:, CD:] = r1["colp"].astype(np.float32) \
        .reshape(B, 2 * NT, M - CD).max(axis=1).astype(np.float16)

    at2 = np.zeros((NCORES, D, NROWS2), np.float32)
    bt2_all = np.zeros((NCORES, 128, GRP * CW), np.float32)
    sg = np.full((NCORES, 128, NST), np.inf, np.float16)
    slot_of_row = np.full((B, N), -1, np.int64)
    core_of_row = np.full((B, N), 0, np.int64)
    overflow = []                                                 # (b, n)
    for b in range(B):
        d1t = desc1[b].T                                          # [128, M]
        for h2 in range(2):
            core = 2 * b + h2
            gchunks = range(h2 * GRP, (h2 + 1) * GRP)
            rows_of = {g: np.nonzero(cstar[b] == g)[0] for g in gchunks}
            # biggest groups get the big slots
            order = sorted(gchunks, key=lambda g: -len(rows_of[g]))
            for slot, g in enumerate(order):
                rws = rows_of[g]
                if len(rws) > SLOTS[slot]:
                    overflow.extend((b, n) for n in rws[SLOTS[slot]:])
                    rws = rws[:SLOTS[slot]]
                slots = SLOT_BASE[slot] + np.arange(len(rws))
                slot_of_row[b, rws] = slots
                core_of_row[b, rws] = core
                at2[core][:, slots] = desc0[b, rws].T
                sg[core][slots % 128, slots // 128] = score0_h[b, rws]
                bt2_all[core][:, slot * CW:(slot + 1) * CW] = \
                    d1t[:, g * CW:(g + 1) * CW]

    sg8 = np.repeat(sg, 8, axis=2)                         # [8,128,NST*8]
    r2 = _run(ex2, {"at2": at2.astype(bf16), "bt2": bt2_all.astype(bf16),
                    "sg": sg8})
    within = r2["idx"][:, :, ::8]                                 # [8, 128, NST]

    sl = np.maximum(slot_of_row, 0)
    cr = core_of_row
    w = within[cr, sl % 128, sl // 128].astype(np.int64)          # [B, N]
    np.clip(w, 0, CW - 1, out=w)
    match01 = (cstar * CW + w).astype(np.int32)
    score0 = score0_h.astype(np.float32)
    valid = (score0 > 0.1) & \
            (score0_h == np.take_along_axis(colmax, match01.astype(np.int64),
                                            axis=1))

    for b, n in overflow:                                         # ~never taken
        simrow = desc0[b, n] @ desc1[b].T
        j = int(simrow.argmax())
        s = simrow.max()
        col = desc0[b] @ desc1[b, j]
        match01[b, n] = j
        score0[b, n] = np.float32(s)
        valid[b, n] = (s > 0.1) & (int(col.argmax()) == n)

    return match01, score0, valid


# revision 84
# speedup vs baseline: 1.0072x; 1.0024x over previous
"""Trainium2 Bass kernel for DescriptorMatcher (mutual nearest neighbor matching).

Problem: given desc0 [B,N,D], desc1 [B,M,D] (B=4, N=M=8192, D=128, fp32):
    sim     = desc0 @ desc1^T                      [B,N,M]
    score0  = max_m sim                            [B,N]
    match01 = argmax_m sim                         [B,N]
    match10 = argmax_n sim                         [B,M]
    valid   = (match10[match01[n]] == n) & (score0 > 0.1)
returns (match01, score0, valid).

Key reformulation: the mutual check never needs match10 indices:
    match10[match01[n]] == n  <=>  score0[n] == colmax[match01[n]]
when all maxima are taken over the SAME rounded values. All on-device
max bookkeeping runs on fp16-rounded copies of the PSUM values: fp16
rounding is monotonic, so max(fp16(v)) == fp16(max(v)) and the
equality trick holds bit-exactly in the fp16 value system. Matmul
inputs ship as bf16 (halves the bandwidth-bound input DMA; the PE
accumulates bf16 products in fp32). Measured argmax flips vs the fp32
reference (bf16 inputs + fp16 value pipeline): 4.8e-3 (gate 2e-2).

Engine constraints that shape the design (walrus BIR verifier): GPSIMD
(Pool) cannot access PSUM and has no tensor_tensor/tensor_scalar ucode
(only partition_all_reduce/memset/ISA lib); TRN2 matmul output must be
fp32; ACT accum is sum-only; DVE fp16 runs tensor_scalar at 4x and
tensor_tensor at 2x (SBUF operands).

Phase 1 (per core = batch x row-half), per 128-row tile [128 x 8192]:
    PE:   16 fp32r matmuls; chunks 0-11 into [128,1536] PSUM tiles
          (bufs=2), chunks 12-15 into [128,512] PSUM tiles (bufs=2)
    ACT:  4x 1536-wide + 1x 512-wide PSUM->SBUF fp16 copies (chunks
          0-12) — ACT is the bottleneck engine at ~100% busy
    DVE:  fused TS copy+chunk-max chunks 13-15 (PSUM src), 13 junk-TS
          chunk-maxes on fp16 (4x mode), colacc[0:CD] = max(.., row)
          (emitted one tile late so pieces are always ready)
    Pool: per-tile partition_all_reduce of row[CD:M] (2 pieces),
          DMA'd out per tile; host folds the 32 per-tile partials
Outputs cm [128, 32*16] fp16 chunk maxima, colacc [128, CD] fp16 and
colp [32, M-CD]; the final column reduces happen on the host.

Phase 2: rows grouped by winning chunk (host); recompute sim[:, chunk]
with identically-laid-out bf16 matmuls -> bit-identical PSUM -> ACT
fp16 copy -> max_index(cm_value, chunk_fp16) = exact first-occurrence
within-chunk argmax. match01 = chunk*512 + within. 34 subtiles with
slot capacities (640x3,512x4,384); the host assigns the largest chunk-
groups to the big slots by permuting bt2/at2 slot contents.

Rows overflowing a slot's capacity fall back to a host recompute.
"""

import numpy as np

import concourse.bass as bass  # noqa: F401  (bass must import before tile)
import concourse.mybir as mybir
import concourse.tile as tile
from concourse import bacc, bass_isa

B, N, M, D = 4, 8192, 8192, 128
NCORES = 8
HALF = N // 2          # rows per phase-1 core
NT = HALF // 128       # 32 n-tiles per core
CW = 512               # chunk width (phase-2 recompute width)
NCHUNK = M // CW       # 16 chunks per row
CD = 3712              # colacc columns on DVE; Pool reduces [CD, M) per tile
GRP = NCHUNK // 2      # 8 chunk-groups per phase-2 core
# Per-core slot capacities (rows) for the 8 chunk-groups: the host assigns
# the largest groups to the big slots (counts are ~N(512, 22), max observed
# 580 on the reference inputs); rows beyond a slot's capacity fall back to
# a host-side recompute.
SLOTS = (640, 640, 640, 512, 512, 512, 512, 384)
SLOT_BASE = tuple(int(x) for x in np.cumsum((0,) + SLOTS)[:-1])
NROWS2 = sum(SLOTS)    # 4608 phase-2 row slots per core
NST = NROWS2 // 128    # 36 phase-2 sub-tiles


def _build1():
    f32 = mybir.dt.float32
    f32r = mybir.dt.float32r
    f16 = mybir.dt.float16
    nc = bacc.Bacc("TRN2", target_bir_lowering=False, debug=False,
                   num_devices=NCORES)
    bf16 = mybir.dt.bfloat16
    at = nc.dram_tensor("at", [D, HALF], bf16, kind="ExternalInput").ap()
    bt = nc.dram_tensor("bt", [D, M], bf16, kind="ExternalInput").ap()
    cm_o = nc.dram_tensor("cm", [128, NT * NCHUNK], f16,
                          kind="ExternalOutput").ap()
    colacc_o = nc.dram_tensor("colacc", [128, CD], f16,
                              kind="ExternalOutput").ap()
    colp_o = nc.dram_tensor("colp", [NT, M - CD], f16,
                            kind="ExternalOutput").ap()

    with tile.TileContext(nc) as tc:
        with tc.tile_pool(name="big", bufs=1) as big, \
             tc.tile_pool(name="rows", bufs=5) as rows, \
             tc.tile_pool(name="dmy", bufs=8) as dmy, \
             tc.tile_pool(name="cps", bufs=3) as cps, \
             tc.tile_pool(name="psa", bufs=2, space="PSUM") as psa, \
             tc.tile_pool(name="psp", bufs=2, space="PSUM") as psp:
            atb = big.tile([128, HALF], bf16, name="atb")
            btb = big.tile([128, M], bf16, name="btb")
            # tile 0 needs at[:, 0:128] and then bt chunks in matmul order;
            # front-load tiny slices on BOTH DGE queues so the PE starts ASAP
            nc.scalar.dma_start(atb[:, 0:128], at[:, 0:128])
            nc.sync.dma_start(btb[:, 0:512], bt[:, 0:512])
            nc.sync.dma_start(btb[:, 512:1024], bt[:, 512:1024])
            nc.scalar.dma_start(btb[:, 1024:2048], bt[:, 1024:2048])
            # rest of bt in wide transfers so tile 0's tail chunks aren't
            # starved behind a long descriptor queue; at after bt.
            for c in range(2048, M, 3072):
                w = min(3072, M - c)
                nc.sync.dma_start(btb[:, c:c + w], bt[:, c:c + w])
            nc.sync.dma_start(atb[:, 128:1024], at[:, 128:1024])
            for c in range(1024, HALF, 3072):
                w = min(3072, HALF - c)
                nc.sync.dma_start(atb[:, c:c + w], at[:, c:c + w])
            cm_all = big.tile([128, NT * NCHUNK], f16, name="cm_all")
            colacc = big.tile([128, M], f16, name="colacc")

            # colacc piece boundaries (moderate quanta: overhead vs blocking)
            dve_cuts = [0, CD]

            def colacc_update(tp, prow):
                """Column-max bookkeeping for row(tp). DVE accumulates
                colacc = max(colacc, row) on [0:CD]; Pool (which cannot
                read PSUM and has no tensor_tensor ucode) instead does a
                per-tile 128-partition max-reduce of row[CD:M], DMA'd out
                for the host to fold over tiles. Emitted one tile LATE so
                the pieces are ready when the engine queues reach them."""
                cuts = dve_cuts
                for lo, hi in zip(cuts[:-1], cuts[1:]):
                    if tp == 0:
                        nc.vector.tensor_copy(colacc[:, lo:hi], prow[:, lo:hi])
                    else:
                        nc.vector.tensor_tensor(colacc[:, lo:hi],
                                                colacc[:, lo:hi],
                                                prow[:, lo:hi],
                                                op=mybir.AluOpType.max)
                    if tp == NT - 1:
                        nc.sync.dma_start(colacc_o[:, lo:hi],
                                          colacc[:, lo:hi])
                # two pieces: the first needs only earlier chunks, so it
                # starts before the tail chunks land (shortens the drain)
                cp = cps.tile([128, M - CD], f16, tag="cp", name="cp")
                pw = (M - CD) // 2
                for q in range(2):
                    lo = CD + q * pw
                    hi = M if q == 1 else lo + pw
                    nc.gpsimd.partition_all_reduce(
                        cp[:, lo - CD:hi - CD], prow[:, lo:hi], channels=128,
                        reduce_op=bass_isa.ReduceOp.max)
                nc.sync.dma_start(colp_o[tp:tp + 1, :], cp[0:1, :])

            def emit_pa(t, h, row, cmt):
                pa = psa.tile([128, 1536], f32, tag="pa", name="pa")
                for j in range(3):
                    mlo = h * 1536 + j * 512
                    nc.tensor.matmul(pa[:, j * 512:(j + 1) * 512],
                                     atb[:, t * 128:(t + 1) * 128],
                                     btb[:, mlo:mlo + 512],
                                     start=True, stop=True)
                # PSUM escape: fp32 -> fp16 on ACT
                nc.scalar.copy(row[:, h * 1536:(h + 1) * 1536], pa[:])
                # chunk maxima for ACT-copied chunks (DVE 4x mode);
                # junk output so colacc only depends on copies.
                dj = dmy.tile([128, 1536], f16, tag="dmy", name="dmy")
                for j in range(3):
                    ch = h * 3 + j
                    nc.vector.tensor_scalar(
                        dj[:, j * CW:(j + 1) * CW],
                        row[:, ch * CW:(ch + 1) * CW], 1.0, None,
                        op0=mybir.AluOpType.mult,
                        op1=mybir.AluOpType.max,
                        accum_out=cmt[:, ch:ch + 1])

            def emit_psp(t, row, cmt):
                for ch in range(12, 16):
                    pp = psp.tile([128, CW], f32, tag="pp", name="pp")
                    nc.tensor.matmul(pp[:],
                                     atb[:, t * 128:(t + 1) * 128],
                                     btb[:, ch * CW:(ch + 1) * CW],
                                     start=True, stop=True)
                    if ch == 12:
                        # chunk 12: ACT copy + DVE junk-TS max
                        nc.scalar.copy(row[:, 12 * CW:13 * CW], pp[:])
                        dj = dmy.tile([128, 1024], f16, tag="dmy", name="dmy")
                        nc.vector.tensor_scalar(
                            dj[:, 0:CW], row[:, 12 * CW:13 * CW], 1.0, None,
                            op0=mybir.AluOpType.mult,
                            op1=mybir.AluOpType.max,
                            accum_out=cmt[:, 12:13])
                    else:
                        # chunks 13-15: DVE-fused copy+chunk-max (PSUM src)
                        nc.vector.tensor_scalar(
                            row[:, ch * CW:(ch + 1) * CW], pp[:],
                            1.0, None,
                            op0=mybir.AluOpType.mult,
                            op1=mybir.AluOpType.max,
                            accum_out=cmt[:, ch:ch + 1])

            prev_row = None
            for t in range(NT):
                row = rows.tile([128, M], f16, tag="row", name="row")
                cmt = cm_all[:, t * NCHUNK:(t + 1) * NCHUNK]
                # Chunks 0-11: PE fills [128,1024] PSUM tiles (bufs=3, so
                # PE runs ahead of the ACT escape copies). GPSIMD cannot
                # read PSUM, so the remaining chunks 12-15 go through psp
                # tiles: chunk 12 ACT-copied, 13-15 DVE-fused copy+max.
                # Last tile: tail chunks first so the final colacc and
                # partition-reduce pieces overlap the last ACT copies.
                if t == NT - 1:
                    emit_psp(t, row, cmt)
                for h in range(4):
                    emit_pa(t, h, row, cmt)
                if t != NT - 1:
                    emit_psp(t, row, cmt)
                # delayed colacc update for the previous tile
                if prev_row is not None:
                    colacc_update(t - 1, prev_row)
                prev_row = row
                # stagger the (tiny) chunk-maxima output
                if t % 8 == 7 and t != NT - 1:
                    nc.sync.dma_start(
                        cm_o[:, (t - 7) * NCHUNK:(t + 1) * NCHUNK],
                        cm_all[:, (t - 7) * NCHUNK:(t + 1) * NCHUNK])
            colacc_update(NT - 1, prev_row)
            nc.sync.dma_start(cm_o[:, (NT - 8) * NCHUNK:],
                              cm_all[:, (NT - 8) * NCHUNK:])
    nc.compile()
    return nc


def _build2():
    f32, f32r, u32 = mybir.dt.float32, mybir.dt.float32r, mybir.dt.uint32
    f16 = mybir.dt.float16
    nc = bacc.Bacc("TRN2", target_bir_lowering=False, debug=False,
                   num_devices=NCORES)
    bf16 = mybir.dt.bfloat16
    at2 = nc.dram_tensor("at2", [D, NROWS2], bf16, kind="ExternalInput").ap()
    bt2 = nc.dram_tensor("bt2", [D, M // 2], bf16, kind="ExternalInput").ap()
    sg = nc.dram_tensor("sg", [128, NST * 8], f16, kind="ExternalInput").ap()
    idx_o = nc.dram_tensor("idx", [128, NST * 8], u32, kind="ExternalOutput").ap()
    with tile.TileContext(nc) as tc:
        with tc.tile_pool(name="big", bufs=1) as big, \
             tc.tile_pool(name="stg", bufs=4) as stg, \
             tc.tile_pool(name="ps", bufs=4, space="PSUM") as ps:
            a2b = big.tile([128, NROWS2], bf16, name="a2b")
            b2b = big.tile([128, M // 2], bf16, name="b2b")
            sgb = big.tile([128, NST * 8], f16, name="sgb")
            # first slices on both DGE queues in parallel
            nc.scalar.dma_start(a2b[:, 0:128], at2[:, 0:128])
            nc.sync.dma_start(b2b[:, 0:512], bt2[:, 0:512])
            nc.scalar.dma_start(sgb[:], sg[:])
            # interleave so group 0's matmuls start before all input lands
            na = (NROWS2 + 1023) // 1024
            nb = (M // 2) // 1024
            for i in range(max(na, nb)):
                if i < na:
                    c = i * 1024
                    lo = 128 if i == 0 else 0
                    w = min(1024, NROWS2 - c)
                    nc.sync.dma_start(a2b[:, c + lo:c + w],
                                      at2[:, c + lo:c + w])
                if i < nb:
                    c = i * 1024
                    lo = 512 if i == 0 else 0
                    nc.sync.dma_start(b2b[:, c + lo:c + 1024],
                                      bt2[:, c + lo:c + 1024])
            idx8 = big.tile([128, NST * 8], u32, name="idx8")
            st = -1
            for g in range(GRP):
                for k in range(SLOTS[g] // 128):
                    st += 1
                    pt = ps.tile([128, CW], f32, tag="pt", name="pt")
                    nc.tensor.matmul(pt[:],
                                     a2b[:, st * 128:(st + 1) * 128],
                                     b2b[:, g * CW:(g + 1) * CW],
                                     start=True, stop=True)
                    ch = stg.tile([128, CW], f16, tag="ch", name="ch")
                    nc.scalar.copy(ch[:], pt[:])
                    nc.vector.max_index(idx8[:, st * 8:(st + 1) * 8],
                                        sgb[:, st * 8:(st + 1) * 8], ch[:])
            nc.sync.dma_start(idx_o[:, 0:(NST - 1) * 8],
                              idx8[:, 0:(NST - 1) * 8])
            nc.sync.dma_start(idx_o[:, (NST - 1) * 8:],
                              idx8[:, (NST - 1) * 8:])
    nc.compile()
    return nc


_cached = None


def _make_exec(nc):
    import jax
    from jax.sharding import Mesh, PartitionSpec
    from jax.experimental.shard_map import shard_map
    from concourse import bass2jax
    from concourse.bass2jax import _bass_exec_p

    partition_name = nc.partition_id_tensor.name if nc.partition_id_tensor else None
    in_names, out_names, out_avals, out_shapes = [], [], [], []
    for alloc in nc.m.functions[0].allocations:
        if not isinstance(alloc, mybir.MemoryLocationSet):
            continue
        name = alloc.memorylocations[0].name
        if alloc.kind == "ExternalInput":
            if name != partition_name:
                in_names.append(name)
        elif alloc.kind == "ExternalOutput":
            shape = tuple(alloc.tensor_shape)
            dtype = mybir.dt.np(alloc.dtype)
            out_names.append(name)
            out_shapes.append((shape, dtype))
            out_avals.append(jax.core.ShapedArray(shape, dtype))
    n_params = len(in_names)
    n_outs = len(out_names)
    all_in_names = in_names + out_names
    if partition_name is not None:
        all_in_names = all_in_names + [partition_name]

    def _body(*args):
        operands = list(args)
        if partition_name is not None:
            operands.append(bass2jax.partition_id_tensor())
        outs = _bass_exec_p.bind(
            *operands, out_avals=tuple(out_avals), in_names=tuple(all_in_names),
            out_names=tuple(out_names), lowering_input_output_aliases=(),
            sim_require_finite=True, sim_require_nnan=True, nc=nc)
        return tuple(outs)

    devices = jax.devices()[:NCORES]
    mesh = Mesh(np.asarray(devices), ("core",))
    in_specs = (PartitionSpec("core"),) * (n_params + n_outs)
    out_specs = (PartitionSpec("core"),) * n_outs
    fn = jax.jit(shard_map(_body, mesh=mesh, in_specs=in_specs,
                           out_specs=out_specs, check_rep=False),
                 keep_unused=True)
    return {"fn": fn, "in_names": in_names, "out_names": out_names,
            "out_shapes": out_shapes, "nc": nc}


def _run(ex, ins):
    """ins: dict name -> [NCORES, *shape]; returns dict name -> [NCORES, *shape]."""
    concat_in = [np.ascontiguousarray(ins[n].reshape(-1, *ins[n].shape[2:]))
                 for n in ex["in_names"]]
    concat_zeros = [np.zeros((NCORES * s[0], *s[1:]), dt)
                    for (s, dt) in ex["out_shapes"]]
    out_arrs = ex["fn"](*concat_in, *concat_zeros)
    return {name: np.asarray(out_arrs[i]).reshape(NCORES, *ex["out_shapes"][i][0])
            for i, name in enumerate(ex["out_names"])}


def kernel(desc0, desc1):
    global _cached
    desc0 = np.asarray(desc0, dtype=np.float32)
    desc1 = np.asarray(desc1, dtype=np.float32)
    assert desc0.shape == (B, N, D) and desc1.shape == (B, M, D)

    if _cached is None:
        _cached = (_make_exec(_build1()), _make_exec(_build2()))
    ex1, ex2 = _cached

    import ml_dtypes
    bf16 = ml_dtypes.bfloat16
    a_slab = np.stack([desc0[b, h * HALF:(h + 1) * HALF]
                       for b in range(B) for h in range(2)])      # [8,4096,128]
    bt_all = np.stack([desc1[b].transpose(1, 0)
                       for b in range(B) for h in range(2)])      # [8,128,8192]
    at_all = a_slab.transpose(0, 2, 1)                            # [8,128,4096]

    r1 = _run(ex1, {"at": at_all.astype(bf16), "bt": bt_all.astype(bf16)})

    # host glue: score/chunk-argmax + column reduce + grouping for phase 2
    cm = r1["cm"].reshape(NCORES, 128, NT, NCHUNK).transpose(0, 2, 1, 3) \
                 .reshape(NCORES, HALF, NCHUNK)
    cm_b32 = cm.reshape(B, N, NCHUNK).astype(np.float32)  # f32: fast numpy
    score0_h = cm_b32.max(axis=2).astype(np.float16)              # [B,N] fp16
    cstar = cm_b32.argmax(axis=2)                                 # [B, N]
    colmax = np.empty((B, M), np.float16)
    colmax[:, :CD] = r1["colacc"].astype(np.float32) \
        .reshape(B, 2 * 128, CD).max(axis=1).astype(np.float16)
    colmax[:, CD:] = r1["colp"].astype(np.float32) \
        .reshape(B, 2 * NT, M - CD).max(axis=1).astype(np.float16)

    at2 = np.zeros((NCORES, D, NROWS2), np.float32)
    bt2_all = np.zeros((NCORES, 128, GRP * CW), np.float32)
    sg = np.full((NCORES, 128, NST), np.inf, np.float16)
    slot_of_row = np.full((B, N), -1, np.int64)
    core_of_row = np.full((B, N), 0, np.int64)
    overflow = []                                                 # (b, n)
    for b in range(B):
        d1t = desc1[b].T                                          # [128, M]
        for h2 in range(2):
            core = 2 * b + h2
            gchunks = range(h2 * GRP, (h2 + 1) * GRP)
            rows_of = {g: np.nonzero(cstar[b] == g)[0] for g in gchunks}
            # biggest groups get the big slots
            order = sorted(gchunks, key=lambda g: -len(rows_of[g]))
            for slot, g in enumerate(order):
                rws = rows_of[g]
                if len(rws) > SLOTS[slot]:
                    overflow.extend((b, n) for n in rws[SLOTS[slot]:])
                    rws = rws[:SLOTS[slot]]
                slots = SLOT_BASE[slot] + np.arange(len(rws))
                slot_of_row[b, rws] = slots
                core_of_row[b, rws] = core
                at2[core][:, slots] = desc0[b, rws].T
                sg[core][slots % 128, slots // 128] = score0_h[b, rws]
                bt2_all[core][:, slot * CW:(slot + 1) * CW] = \
                    d1t[:, g * CW:(g + 1) * CW]

    sg8 = np.repeat(sg, 8, axis=2)                         # [8,128,NST*8]
    r2 = _run(ex2, {"at2": at2.astype(bf16), "bt2": bt2_all.astype(bf16),
                    "sg": sg8})
    within = r2["idx"][:, :, ::8]                                 # [8, 128, NST]

    sl = np.maximum(slot_of_row, 0)
    cr = core_of_row
    w = within[cr, sl % 128, sl // 128].astype(np.int64)          # [B, N]
    np.clip(w, 0, CW - 1, out=w)
    match01 = (cstar * CW + w).astype(np.int32)
    score0 = score0_h.astype(np.float32)
    valid = (score0 > 0.1) & \
            (score0_h == np.take_along_axis(colmax, match01.astype(np.int64),
                                            axis=1))

    if overflow:                       # rows beyond slot capacity: exact host path
        from collections import defaultdict
        byb = defaultdict(list)
        for b, n in overflow:
            byb[b].append(n)
        for b, ns in byb.items():
            ns = np.asarray(ns)
            simr = desc0[b, ns] @ desc1[b].T              # [k, M]
            js = simr.argmax(1)
            cols = desc1[b, js] @ desc0[b].T              # [k, N]
            match01[b, ns] = js.astype(np.int32)
            score0[b, ns] = simr.max(1)
            valid[b, ns] = (simr.max(1) > 0.1) & (cols.argmax(1) == ns)

    return match01, score0, valid


# revision 85
# speedup vs baseline: 1.0097x; 1.0024x over previous
"""Trainium2 Bass kernel for DescriptorMatcher (mutual nearest neighbor matching).

Problem: given desc0 [B,N,D], desc1 [B,M,D] (B=4, N=M=8192, D=128, fp32):
    sim     = desc0 @ desc1^T                      [B,N,M]
    score0  = max_m sim                            [B,N]
    match01 = argmax_m sim                         [B,N]
    match10 = argmax_n sim                         [B,M]
    valid   = (match10[match01[n]] == n) & (score0 > 0.1)
returns (match01, score0, valid).

Key reformulation: the mutual check never needs match10 indices:
    match10[match01[n]] == n  <=>  score0[n] == colmax[match01[n]]
when all maxima are taken over the SAME rounded values. All on-device
max bookkeeping runs on fp16-rounded copies of the PSUM values: fp16
rounding is monotonic, so max(fp16(v)) == fp16(max(v)) and the
equality trick holds bit-exactly in the fp16 value system. Matmul
inputs ship as bf16 (halves the bandwidth-bound input DMA; the PE
accumulates bf16 products in fp32). Measured argmax flips vs the fp32
reference (bf16 inputs + fp16 value pipeline): 4.8e-3 (gate 2e-2).

Engine constraints that shape the design (walrus BIR verifier): GPSIMD
(Pool) cannot access PSUM and has no tensor_tensor/tensor_scalar ucode
(only partition_all_reduce/memset/ISA lib); TRN2 matmul output must be
fp32; ACT accum is sum-only; DVE fp16 runs tensor_scalar at 4x and
tensor_tensor at 2x (SBUF operands).

Phase 1 (per core = batch x row-half), per 128-row tile [128 x 8192]:
    PE:   16 fp32r matmuls; chunks 0-11 into [128,1536] PSUM tiles
          (bufs=2), chunks 12-15 into [128,512] PSUM tiles (bufs=2)
    ACT:  4x 1536-wide + 1x 512-wide PSUM->SBUF fp16 copies (chunks
          0-12) — ACT is the bottleneck engine at ~100% busy
    DVE:  fused TS copy+chunk-max chunks 13-15 (PSUM src), 13 junk-TS
          chunk-maxes on fp16 (4x mode), colacc[0:CD] = max(.., row)
          (emitted one tile late so pieces are always ready)
    Pool: per-tile partition_all_reduce of row[CD:M] (2 pieces),
          DMA'd out per tile; host folds the 32 per-tile partials
Outputs cm [128, 32*16] fp16 chunk maxima, colacc [128, CD] fp16 and
colp [32, M-CD]; the final column reduces happen on the host.

Phase 2: rows grouped by winning chunk (host); recompute sim[:, chunk]
with identically-laid-out bf16 matmuls -> bit-identical PSUM -> ACT
fp16 copy -> max_index(cm_value, chunk_fp16) = exact first-occurrence
within-chunk argmax. match01 = chunk*512 + within. 33 subtiles with
slot capacities (640x3,512x3,384x2); the host assigns the largest chunk-
groups to the big slots by permuting bt2/at2 slot contents.

Rows overflowing a slot's capacity fall back to a host recompute.
"""

import numpy as np

import concourse.bass as bass  # noqa: F401  (bass must import before tile)
import concourse.mybir as mybir
import concourse.tile as tile
from concourse import bacc, bass_isa

B, N, M, D = 4, 8192, 8192, 128
NCORES = 8
HALF = N // 2          # rows per phase-1 core
NT = HALF // 128       # 32 n-tiles per core
CW = 512               # chunk width (phase-2 recompute width)
NCHUNK = M // CW       # 16 chunks per row
CD = 3712              # colacc columns on DVE; Pool reduces [CD, M) per tile
GRP = NCHUNK // 2      # 8 chunk-groups per phase-2 core
# Per-core slot capacities (rows) for the 8 chunk-groups: the host assigns
# the largest groups to the big slots (counts are ~N(512, 22), max observed
# 580 on the reference inputs); rows beyond a slot's capacity fall back to
# a host-side recompute.
SLOTS = (640, 640, 640, 512, 512, 512, 384, 384)
SLOT_BASE = tuple(int(x) for x in np.cumsum((0,) + SLOTS)[:-1])
NROWS2 = sum(SLOTS)    # 4608 phase-2 row slots per core
NST = NROWS2 // 128    # 36 phase-2 sub-tiles


def _build1():
    f32 = mybir.dt.float32
    f32r = mybir.dt.float32r
    f16 = mybir.dt.float16
    nc = bacc.Bacc("TRN2", target_bir_lowering=False, debug=False,
                   num_devices=NCORES)
    bf16 = mybir.dt.bfloat16
    at = nc.dram_tensor("at", [D, HALF], bf16, kind="ExternalInput").ap()
    bt = nc.dram_tensor("bt", [D, M], bf16, kind="ExternalInput").ap()
    cm_o = nc.dram_tensor("cm", [128, NT * NCHUNK], f16,
                          kind="ExternalOutput").ap()
    colacc_o = nc.dram_tensor("colacc", [128, CD], f16,
                              kind="ExternalOutput").ap()
    colp_o = nc.dram_tensor("colp", [NT, M - CD], f16,
                            kind="ExternalOutput").ap()

    with tile.TileContext(nc) as tc:
        with tc.tile_pool(name="big", bufs=1) as big, \
             tc.tile_pool(name="rows", bufs=5) as rows, \
             tc.tile_pool(name="dmy", bufs=8) as dmy, \
             tc.tile_pool(name="cps", bufs=3) as cps, \
             tc.tile_pool(name="psa", bufs=2, space="PSUM") as psa, \
             tc.tile_pool(name="psp", bufs=2, space="PSUM") as psp:
            atb = big.tile([128, HALF], bf16, name="atb")
            btb = big.tile([128, M], bf16, name="btb")
            # tile 0 needs at[:, 0:128] and then bt chunks in matmul order;
            # front-load tiny slices on BOTH DGE queues so the PE starts ASAP
            nc.scalar.dma_start(atb[:, 0:128], at[:, 0:128])
            nc.sync.dma_start(btb[:, 0:512], bt[:, 0:512])
            nc.sync.dma_start(btb[:, 512:1024], bt[:, 512:1024])
            nc.scalar.dma_start(btb[:, 1024:2048], bt[:, 1024:2048])
            # rest of bt in wide transfers so tile 0's tail chunks aren't
            # starved behind a long descriptor queue; at after bt.
            for c in range(2048, M, 3072):
                w = min(3072, M - c)
                nc.sync.dma_start(btb[:, c:c + w], bt[:, c:c + w])
            nc.sync.dma_start(atb[:, 128:1024], at[:, 128:1024])
            for c in range(1024, HALF, 3072):
                w = min(3072, HALF - c)
                nc.sync.dma_start(atb[:, c:c + w], at[:, c:c + w])
            cm_all = big.tile([128, NT * NCHUNK], f16, name="cm_all")
            colacc = big.tile([128, M], f16, name="colacc")

            # colacc piece boundaries (moderate quanta: overhead vs blocking)
            dve_cuts = [0, CD]

            def colacc_update(tp, prow):
                """Column-max bookkeeping for row(tp). DVE accumulates
                colacc = max(colacc, row) on [0:CD]; Pool (which cannot
                read PSUM and has no tensor_tensor ucode) instead does a
                per-tile 128-partition max-reduce of row[CD:M], DMA'd out
                for the host to fold over tiles. Emitted one tile LATE so
                the pieces are ready when the engine queues reach them."""
                cuts = dve_cuts
                for lo, hi in zip(cuts[:-1], cuts[1:]):
                    if tp == 0:
                        nc.vector.tensor_copy(colacc[:, lo:hi], prow[:, lo:hi])
                    else:
                        nc.vector.tensor_tensor(colacc[:, lo:hi],
                                                colacc[:, lo:hi],
                                                prow[:, lo:hi],
                                                op=mybir.AluOpType.max)
                    if tp == NT - 1:
                        nc.sync.dma_start(colacc_o[:, lo:hi],
                                          colacc[:, lo:hi])
                # two pieces: the first needs only earlier chunks, so it
                # starts before the tail chunks land (shortens the drain)
                cp = cps.tile([128, M - CD], f16, tag="cp", name="cp")
                pw = (M - CD) // 2
                for q in range(2):
                    lo = CD + q * pw
                    hi = M if q == 1 else lo + pw
                    nc.gpsimd.partition_all_reduce(
                        cp[:, lo - CD:hi - CD], prow[:, lo:hi], channels=128,
                        reduce_op=bass_isa.ReduceOp.max)
                nc.sync.dma_start(colp_o[tp:tp + 1, :], cp[0:1, :])

            def emit_pa(t, h, row, cmt):
                pa = psa.tile([128, 1536], f32, tag="pa", name="pa")
                for j in range(3):
                    mlo = h * 1536 + j * 512
                    nc.tensor.matmul(pa[:, j * 512:(j + 1) * 512],
                                     atb[:, t * 128:(t + 1) * 128],
                                     btb[:, mlo:mlo + 512],
                                     start=True, stop=True)
                # PSUM escape: fp32 -> fp16 on ACT
                nc.scalar.copy(row[:, h * 1536:(h + 1) * 1536], pa[:])
                # chunk maxima for ACT-copied chunks (DVE 4x mode);
                # junk output so colacc only depends on copies.
                dj = dmy.tile([128, 1536], f16, tag="dmy", name="dmy")
                for j in range(3):
                    ch = h * 3 + j
                    nc.vector.tensor_scalar(
                        dj[:, j * CW:(j + 1) * CW],
                        row[:, ch * CW:(ch + 1) * CW], 1.0, None,
                        op0=mybir.AluOpType.mult,
                        op1=mybir.AluOpType.max,
                        accum_out=cmt[:, ch:ch + 1])

            def emit_psp(t, row, cmt):
                for ch in range(12, 16):
                    pp = psp.tile([128, CW], f32, tag="pp", name="pp")
                    nc.tensor.matmul(pp[:],
                                     atb[:, t * 128:(t + 1) * 128],
                                     btb[:, ch * CW:(ch + 1) * CW],
                                     start=True, stop=True)
                    if ch == 12:
                        # chunk 12: ACT copy + DVE junk-TS max
                        nc.scalar.copy(row[:, 12 * CW:13 * CW], pp[:])
                        dj = dmy.tile([128, 1024], f16, tag="dmy", name="dmy")
                        nc.vector.tensor_scalar(
                            dj[:, 0:CW], row[:, 12 * CW:13 * CW], 1.0, None,
                            op0=mybir.AluOpType.mult,
                            op1=mybir.AluOpType.max,
                            accum_out=cmt[:, 12:13])
                    else:
                        # chunks 13-15: DVE-fused copy+chunk-max (PSUM src)
                        nc.vector.tensor_scalar(
                            row[:, ch * CW:(ch + 1) * CW], pp[:],
                            1.0, None,
                            op0=mybir.AluOpType.mult,
                            op1=mybir.AluOpType.max,
                            accum_out=cmt[:, ch:ch + 1])

            prev_row = None
            for t in range(NT):
                row = rows.tile([128, M], f16, tag="row", name="row")
                cmt = cm_all[:, t * NCHUNK:(t + 1) * NCHUNK]
                # Chunks 0-11: PE fills [128,1024] PSUM tiles (bufs=3, so
                # PE runs ahead of the ACT escape copies). GPSIMD cannot
                # read PSUM, so the remaining chunks 12-15 go through psp
                # tiles: chunk 12 ACT-copied, 13-15 DVE-fused copy+max.
                # Last tile: tail chunks first so the final colacc and
                # partition-reduce pieces overlap the last ACT copies.
                if t == NT - 1:
                    emit_psp(t, row, cmt)
                for h in range(4):
                    emit_pa(t, h, row, cmt)
                if t != NT - 1:
                    emit_psp(t, row, cmt)
                # delayed colacc update for the previous tile
                if prev_row is not None:
                    colacc_update(t - 1, prev_row)
                prev_row = row
                # stagger the (tiny) chunk-maxima output
                if t % 8 == 7 and t != NT - 1:
                    nc.sync.dma_start(
                        cm_o[:, (t - 7) * NCHUNK:(t + 1) * NCHUNK],
                        cm_all[:, (t - 7) * NCHUNK:(t + 1) * NCHUNK])
            colacc_update(NT - 1, prev_row)
            nc.sync.dma_start(cm_o[:, (NT - 8) * NCHUNK:],
                              cm_all[:, (NT - 8) * NCHUNK:])
    nc.compile()
    return nc


def _build2():
    f32, f32r, u32 = mybir.dt.float32, mybir.dt.float32r, mybir.dt.uint32
    f16 = mybir.dt.float16
    nc = bacc.Bacc("TRN2", target_bir_lowering=False, debug=False,
                   num_devices=NCORES)
    bf16 = mybir.dt.bfloat16
    at2 = nc.dram_tensor("at2", [D, NROWS2], bf16, kind="ExternalInput").ap()
    bt2 = nc.dram_tensor("bt2", [D, M // 2], bf16, kind="ExternalInput").ap()
    sg = nc.dram_tensor("sg", [128, NST * 8], f16, kind="ExternalInput").ap()
    idx_o = nc.dram_tensor("idx", [128, NST * 8], u32, kind="ExternalOutput").ap()
    with tile.TileContext(nc) as tc:
        with tc.tile_pool(name="big", bufs=1) as big, \
             tc.tile_pool(name="stg", bufs=4) as stg, \
             tc.tile_pool(name="ps", bufs=4, space="PSUM") as ps:
            a2b = big.tile([128, NROWS2], bf16, name="a2b")
            b2b = big.tile([128, M // 2], bf16, name="b2b")
            sgb = big.tile([128, NST * 8], f16, name="sgb")
            # first slices on both DGE queues in parallel
            nc.scalar.dma_start(a2b[:, 0:128], at2[:, 0:128])
            nc.sync.dma_start(b2b[:, 0:512], bt2[:, 0:512])
            nc.scalar.dma_start(sgb[:], sg[:])
            # interleave so group 0's matmuls start before all input lands
            na = (NROWS2 + 1023) // 1024
            nb = (M // 2) // 1024
            for i in range(max(na, nb)):
                if i < na:
                    c = i * 1024
                    lo = 128 if i == 0 else 0
                    w = min(1024, NROWS2 - c)
                    nc.sync.dma_start(a2b[:, c + lo:c + w],
                                      at2[:, c + lo:c + w])
                if i < nb:
                    c = i * 1024
                    lo = 512 if i == 0 else 0
                    nc.sync.dma_start(b2b[:, c + lo:c + 1024],
                                      bt2[:, c + lo:c + 1024])
            idx8 = big.tile([128, NST * 8], u32, name="idx8")
            st = -1
            for g in range(GRP):
                for k in range(SLOTS[g] // 128):
                    st += 1
                    pt = ps.tile([128, CW], f32, tag="pt", name="pt")
                    nc.tensor.matmul(pt[:],
                                     a2b[:, st * 128:(st + 1) * 128],
                                     b2b[:, g * CW:(g + 1) * CW],
                                     start=True, stop=True)
                    ch = stg.tile([128, CW], f16, tag="ch", name="ch")
                    nc.scalar.copy(ch[:], pt[:])
                    nc.vector.max_index(idx8[:, st * 8:(st + 1) * 8],
                                        sgb[:, st * 8:(st + 1) * 8], ch[:])
            nc.sync.dma_start(idx_o[:, 0:(NST - 1) * 8],
                              idx8[:, 0:(NST - 1) * 8])
            nc.sync.dma_start(idx_o[:, (NST - 1) * 8:],
                              idx8[:, (NST - 1) * 8:])
    nc.compile()
    return nc


_cached = None


def _make_exec(nc):
    import jax
    from jax.sharding import Mesh, PartitionSpec
    from jax.experimental.shard_map import shard_map
    from concourse import bass2jax
    from concourse.bass2jax import _bass_exec_p

    partition_name = nc.partition_id_tensor.name if nc.partition_id_tensor else None
    in_names, out_names, out_avals, out_shapes = [], [], [], []
    for alloc in nc.m.functions[0].allocations:
        if not isinstance(alloc, mybir.MemoryLocationSet):
            continue
        name = alloc.memorylocations[0].name
        if alloc.kind == "ExternalInput":
            if name != partition_name:
                in_names.append(name)
        elif alloc.kind == "ExternalOutput":
            shape = tuple(alloc.tensor_shape)
            dtype = mybir.dt.np(alloc.dtype)
            out_names.append(name)
            out_shapes.append((shape, dtype))
            out_avals.append(jax.core.ShapedArray(shape, dtype))
    n_params = len(in_names)
    n_outs = len(out_names)
    all_in_names = in_names + out_names
    if partition_name is not None:
        all_in_names = all_in_names + [partition_name]

    def _body(*args):
        operands = list(args)
        if partition_name is not None:
            operands.append(bass2jax.partition_id_tensor())
        outs = _bass_exec_p.bind(
            *operands, out_avals=tuple(out_avals), in_names=tuple(all_in_names),
            out_names=tuple(out_names), lowering_input_output_aliases=(),
            sim_require_finite=True, sim_require_nnan=True, nc=nc)
        return tuple(outs)

    devices = jax.devices()[:NCORES]
    mesh = Mesh(np.asarray(devices), ("core",))
    in_specs = (PartitionSpec("core"),) * (n_params + n_outs)
    out_specs = (PartitionSpec("core"),) * n_outs
    fn = jax.jit(shard_map(_body, mesh=mesh, in_specs=in_specs,
                           out_specs=out_specs, check_rep=False),
                 keep_unused=True)
    return {"fn": fn, "in_names": in_names, "out_names": out_names,
            "out_shapes": out_shapes, "nc": nc}


def _run(ex, ins):
    """ins: dict name -> [NCORES, *shape]; returns dict name -> [NCORES, *shape]."""
    concat_in = [np.ascontiguousarray(ins[n].reshape(-1, *ins[n].shape[2:]))
                 for n in ex["in_names"]]
    concat_zeros = [np.zeros((NCORES * s[0], *s[1:]), dt)
                    for (s, dt) in ex["out_shapes"]]
    out_arrs = ex["fn"](*concat_in, *concat_zeros)
    return {name: np.asarray(out_arrs[i]).reshape(NCORES, *ex["out_shapes"][i][0])
            for i, name in enumerate(ex["out_names"])}


def kernel(desc0, desc1):
    global _cached
    desc0 = np.asarray(desc0, dtype=np.float32)
    desc1 = np.asarray(desc1, dtype=np.float32)
    assert desc0.shape == (B, N, D) and desc1.shape == (B, M, D)

    if _cached is None:
        _cached = (_make_exec(_build1()), _make_exec(_build2()))
    ex1, ex2 = _cached

    import ml_dtypes
    bf16 = ml_dtypes.bfloat16
    a_slab = np.stack([desc0[b, h * HALF:(h + 1) * HALF]
                       for b in range(B) for h in range(2)])      # [8,4096,128]
    bt_all = np.stack([desc1[b].transpose(1, 0)
                       for b in range(B) for h in range(2)])      # [8,128,8192]
    at_all = a_slab.transpose(0, 2, 1)                            # [8,128,4096]

    r1 = _run(ex1, {"at": at_all.astype(bf16), "bt": bt_all.astype(bf16)})

    # host glue: score/chunk-argmax + column reduce + grouping for phase 2
    cm = r1["cm"].reshape(NCORES, 128, NT, NCHUNK).transpose(0, 2, 1, 3) \
                 .reshape(NCORES, HALF, NCHUNK)
    cm_b32 = cm.reshape(B, N, NCHUNK).astype(np.float32)  # f32: fast numpy
    score0_h = cm_b32.max(axis=2).astype(np.float16)              # [B,N] fp16
    cstar = cm_b32.argmax(axis=2)                                 # [B, N]
    colmax = np.empty((B, M), np.float16)
    colmax[:, :CD] = r1["colacc"].astype(np.float32) \
        .reshape(B, 2 * 128, CD).max(axis=1).astype(np.float16)
    colmax[:, CD:] = r1["colp"].astype(np.float32) \
        .reshape(B, 2 * NT, M - CD).max(axis=1).astype(np.float16)

    at2 = np.zeros((NCORES, D, NROWS2), np.float32)
    bt2_all = np.zeros((NCORES, 128, GRP * CW), np.float32)
    sg = np.full((NCORES, 128, NST), np.inf, np.float16)
    slot_of_row = np.full((B, N), -1, np.int64)
    core_of_row = np.full((B, N), 0, np.int64)
    overflow = []                                                 # (b, n)
    for b in range(B):
        d1t = desc1[b].T                                          # [128, M]
        for h2 in range(2):
            core = 2 * b + h2
            gchunks = range(h2 * GRP, (h2 + 1) * GRP)
            rows_of = {g: np.nonzero(cstar[b] == g)[0] for g in gchunks}
            # biggest groups get the big slots
            order = sorted(gchunks, key=lambda g: -len(rows_of[g]))
            for slot, g in enumerate(order):
                rws = rows_of[g]
                if len(rws) > SLOTS[slot]:
                    overflow.extend((b, n) for n in rws[SLOTS[slot]:])
                    rws = rws[:SLOTS[slot]]
                slots = SLOT_BASE[slot] + np.arange(len(rws))
                slot_of_row[b, rws] = slots
                core_of_row[b, rws] = core
                at2[core][:, slots] = desc0[b, rws].T
                sg[core][slots % 128, slots // 128] = score0_h[b, rws]
                bt2_all[core][:, slot * CW:(slot + 1) * CW] = \
                    d1t[:, g * CW:(g + 1) * CW]

    sg8 = np.repeat(sg, 8, axis=2)                         # [8,128,NST*8]
    r2 = _run(ex2, {"at2": at2.astype(bf16), "bt2": bt2_all.astype(bf16),
                    "sg": sg8})
    within = r2["idx"][:, :, ::8]                                 # [8, 128, NST]

    sl = np.maximum(slot_of_row, 0)
    cr = core_of_row
    w = within[cr, sl % 128, sl // 128].astype(np.int64)          # [B, N]
    np.clip(w, 0, CW - 1, out=w)
    match01 = (cstar * CW + w).astype(np.int32)
    score0 = score0_h.astype(np.float32)
    valid = (score0 > 0.1) & \
            (score0_h == np.take_along_axis(colmax, match01.astype(np.int64),
                                            axis=1))

    if overflow:                       # rows beyond slot capacity: exact host path
        from collections import defaultdict
        byb = defaultdict(list)
        for b, n in overflow:
            byb[b].append(n)
        for b, ns in byb.items():
            ns = np.asarray(ns)
            simr = desc0[b, ns] @ desc1[b].T              # [k, M]
            js = simr.argmax(1)
            cols = desc1[b, js] @ desc0[b].T              # [k, N]
            match01[b, ns] = js.astype(np.int32)
            score0[b, ns] = simr.max(1)
            valid[b, ns] = (simr.max(1) > 0.1) & (cols.argmax(1) == ns)

    return match01, score0, valid
